# revision 1
# baseline (speedup 1.0000x reference)
"""ACAN sparse-attention kernel for 8x TRN2 NeuronCores.

Data-parallel over batch n=8 (one batch image per core, weights replicated).
Per core: x [16384, 256] instance-norm + QK self/cross scores vs 18 text
tokens + per-pixel softmax(19) + weighted mu/sigma modulation.

Math (per pixel p, channel c):
  out = (1 + ws)*xn + wm + sx*x
      = xn + rz*(Q (.) x) + rz*P + sx*x
  where  E   = exp([x_q@t_k.T | x_q.x_k])   (unnormalized, rz = 1/sum E)
         P   = E[:18].T @ (mu_t - sigma_t*(mu*rs))   [PSUM matmul]
         Q   = E[:18].T @ (sigma_t*rs)               [PSUM matmul]
         xn  = (x - mu)*rs
All big matmuls run as float32r (full-rate fp32 on the PE for N>=256).
"""

import os
import sys

import numpy as np

for _p in ("/opt/trn_rl_repo", "/root/.axon_site/_ro/trn_rl_repo"):
    if os.path.isdir(_p) and _p not in sys.path:
        sys.path.append(_p)

import concourse.bass as bass
import concourse.bacc as bacc
import concourse.tile as tile
from concourse import mybir
from concourse.bass_utils import run_bass_kernel_spmd

F32 = mybir.dt.float32
F32R = mybir.dt.float32r
AF = mybir.ActivationFunctionType
OP = mybir.AluOpType

P = 128          # partitions
C = 256          # channels
DK = 128         # query/key dim
LK = 18          # text tokens
NPIX = 128 * 128
TW = 512         # pixels per phase-2 tile
NT = NPIX // TW  # 32 tiles
CHUNK = 1024     # pixels per phase-1 DMA chunk
NG = NPIX // CHUNK  # 16 chunks
EPS = 1e-5

LAST_RESULT = None


def rr(ap):
    return ap.bitcast(F32R)


def _build(nc: bass.Bass):
    x_ext = nc.declare_dram_parameter("x", [NPIX, C], F32, isOutput=False)
    t_ext = nc.declare_dram_parameter("t", [LK, C], F32, isOutput=False)
    wq_ext = nc.declare_dram_parameter("Wq", [C, DK], F32, isOutput=False)
    bq_ext = nc.declare_dram_parameter("bq", [DK, 1], F32, isOutput=False)
    wk_ext = nc.declare_dram_parameter("Wk", [C, DK], F32, isOutput=False)
    bk_ext = nc.declare_dram_parameter("bk", [DK, 1], F32, isOutput=False)
    wtk_ext = nc.declare_dram_parameter("Wtk", [C, DK], F32, isOutput=False)
    btk_ext = nc.declare_dram_parameter("btk", [1, DK], F32, isOutput=False)
    wmu_ext = nc.declare_dram_parameter("Wmu", [C, C], F32, isOutput=False)
    bmu_ext = nc.declare_dram_parameter("bmu", [1, C], F32, isOutput=False)
    wsig_ext = nc.declare_dram_parameter("Wsig", [C, C], F32, isOutput=False)
    bsig_ext = nc.declare_dram_parameter("bsig", [1, C], F32, isOutput=False)
    out_ext = nc.declare_dram_parameter("out", [NPIX, C], F32, isOutput=True)

    with tile.TileContext(nc) as tc:
        with (
            tc.tile_pool(name="consts", bufs=1) as consts,
            tc.tile_pool(name="work", bufs=2) as work,
            tc.tile_pool(name="sq", bufs=1) as sqpool,
            tc.tile_pool(name="dd", bufs=4) as ddpool,
            tc.tile_pool(name="outp", bufs=2) as outp,
            tc.tile_pool(name="pstr", bufs=2, space="PSUM") as pstr,
            tc.tile_pool(name="psmm", bufs=2, space="PSUM") as psmm,
            tc.tile_pool(name="psp", bufs=2, space="PSUM") as psp,
            tc.tile_pool(name="psq", bufs=2, space="PSUM") as psq,
        ):
            # ---------------- constants / small inputs ----------------
            from concourse.masks import make_identity
            ident_f = consts.tile([P, P], F32, tag="identf")
            make_identity(nc, ident_f)
            ident = consts.tile([P, P], F32R, tag="ident")
            nc.scalar.activation(ident, ident_f, AF.Copy)
            ones2_f = consts.tile([P, 2], F32, tag="ones2f")
            nc.vector.memset(ones2_f, 1.0)
            ones2 = consts.tile([P, 2], F32R, tag="ones2")
            nc.scalar.activation(ones2, ones2_f, AF.Copy)
            ones_col = ones2[:, :1]
            onesr_f = consts.tile([1, P], F32, tag="onesrf")
            nc.vector.memset(onesr_f, 1.0)
            ones_row = consts.tile([1, P], F32R, tag="onesr")
            nc.scalar.activation(ones_row, onesr_f, AF.Copy)
            eps1 = consts.tile([1, 1], F32, tag="eps")
            nc.vector.memset(eps1, EPS)

            wq_sb = consts.tile([P, 2, DK], F32R, tag="wq")
            nc.sync.dma_start(out=wq_sb, in_=wq_ext.rearrange("(h p) d -> p h d", p=P).bitcast(F32R))
            wk_sb = consts.tile([P, 2, DK], F32R, tag="wk")
            nc.sync.dma_start(out=wk_sb, in_=wk_ext.rearrange("(h p) d -> p h d", p=P).bitcast(F32R))
            wtk_sb = consts.tile([P, 2, DK], F32R, tag="wtk")
            nc.sync.dma_start(out=wtk_sb, in_=wtk_ext.rearrange("(h p) d -> p h d", p=P).bitcast(F32R))
            wmu_sb = consts.tile([P, 2, C], F32R, tag="wmu")
            nc.sync.dma_start(out=wmu_sb, in_=wmu_ext.rearrange("(h p) d -> p h d", p=P).bitcast(F32R))
            wsig_sb = consts.tile([P, 2, C], F32R, tag="wsig")
            nc.sync.dma_start(out=wsig_sb, in_=wsig_ext.rearrange("(h p) d -> p h d", p=P).bitcast(F32R))
            bq_sb = consts.tile([P, 1], F32, tag="bq")
            nc.sync.dma_start(out=bq_sb, in_=bq_ext[:, :])
            bk_sb = consts.tile([P, 1], F32, tag="bk")
            nc.sync.dma_start(out=bk_sb, in_=bk_ext[:, :])
            btk_sb = consts.tile([1, DK], F32R, tag="btk")
            nc.sync.dma_start(out=btk_sb, in_=btk_ext[:, :].bitcast(F32R))
            bmu_sb = consts.tile([1, C], F32R, tag="bmu")
            nc.sync.dma_start(out=bmu_sb, in_=bmu_ext[:, :].bitcast(F32R))
            bsig_sb = consts.tile([1, C], F32R, tag="bsig")
            nc.sync.dma_start(out=bsig_sb, in_=bsig_ext[:, :].bitcast(F32R))
            t_sb = consts.tile([P, C], F32R, tag="tsb")
            nc.sync.dma_start(out=t_sb[:LK, :], in_=t_ext[:, :].bitcast(F32R))

            # resident pixel-major x: free index = tile128*C + c
            X = consts.tile([P, NPIX // P * C], F32R, tag="X")

            # ---------------- phase 0a: text projections ----------------
            # tT (channel-major t) via PE transpose
            trT = pstr.tile([P, TW], F32, tag="tr")
            for h in range(2):
                nc.tensor.transpose(
                    rr(trT[:, h * 32 : h * 32 + LK]),
                    rr(t_sb[:LK, h * P : (h + 1) * P]),
                    rr(ident[:LK, :LK]),
                )
            tT_sb = consts.tile([P, 64], F32R, tag="tT")
            nc.scalar.activation(tT_sb, trT[:, :64], AF.Copy)

            # t_kT [dk, 18] = Wtk.T @ tT + btk
            tkp = psmm.tile([P, TW], F32, tag="mm")
            for h in range(2):
                nc.tensor.matmul(
                    tkp[:, :LK],
                    rr(wtk_sb[:, h, :]),
                    rr(tT_sb[:, h * 32 : h * 32 + LK]),
                    start=(h == 0),
                    stop=False,
                )
            nc.tensor.matmul(
                tkp[:, :LK], rr(btk_sb), rr(ones_row[:, :LK]), start=False, stop=True
            )
            tkT_sb = consts.tile([P, 32], F32R, tag="tkT")
            nc.scalar.activation(tkT_sb[:, :LK], tkp[:, :LK], AF.Copy)

            # mu_t / sigma_t [18, C]
            mu_t = consts.tile([P, C], F32, tag="mut")
            sig_t = consts.tile([P, C], F32, tag="sigt")
            for dst, wsb, brow in ((mu_t, wmu_sb, bmu_sb), (sig_t, wsig_sb, bsig_sb)):
                pp = psp.tile([P, C], F32, tag="p")
                for h in range(2):
                    nc.tensor.matmul(
                        pp[:LK, :],
                        rr(tT_sb[:, h * 32 : h * 32 + LK]),
                        rr(wsb[:, h, :]),
                        start=(h == 0),
                        stop=False,
                    )
                nc.tensor.matmul(
                    pp[:LK, :], rr(ones_row[:, :LK]), rr(brow), start=False, stop=True
                )
                nc.scalar.activation(dst[:LK, :], pp[:LK, :], AF.Copy)

            # ---------------- phase 1: stream x in, accumulate stats ----------------
            s1 = psmm.tile([1, TW], F32, tag="mm")   # raw sums per (parity,c)
            s2 = psmm.tile([1, TW], F32, tag="mm")  # square sums
            for g in range(NG):
                xg = X[:, g * (CHUNK // P) * C : (g + 1) * (CHUNK // P) * C]
                nc.sync.dma_start(
                    out=xg.rearrange("p (t c) -> p t c", c=C),
                    in_=x_ext[g * CHUNK : (g + 1) * CHUNK, :].rearrange(
                        "(t p) c -> p t c", p=P
                    ).bitcast(F32R),
                )
                xsq = sqpool.tile([P, (CHUNK // P) * C], F32R, tag="xsq")
                nc.vector.tensor_mul(xsq, xg, xg)
                xc = sqpool.tile([P, (CHUNK // P) * C], F32R, tag="xc")
                nc.scalar.activation(xc, xg, AF.Copy)
                for j in range(4):
                    nc.tensor.matmul(
                        s1,
                        rr(ones_col),
                        rr(xc[:, j * TW : (j + 1) * TW]),
                        start=(g == 0 and j == 0),
                        stop=(g == NG - 1 and j == 3),
                    )
                    nc.tensor.matmul(
                        s2,
                        rr(ones_col),
                        rr(xsq[:, j * TW : (j + 1) * TW]),
                        start=(g == 0 and j == 0),
                        stop=(g == NG - 1 and j == 3),
                    )

            # ---------------- phase 0b: finalize stats + fold tensors ----------------
            s1s = work.tile([1, TW], F32, tag="srowl")
            nc.scalar.activation(s1s, s1, AF.Copy)
            s2s = work.tile([1, TW], F32, tag="srowl")
            nc.scalar.activation(s2s, s2, AF.Copy)
            mean_r = consts.tile([1, C], F32R, tag="meanr")
            nc.vector.tensor_add(mean_r, s1s[:, :C], s1s[:, C:])
            nc.vector.tensor_scalar_mul(mean_r, mean_r, 1.0 / NPIX)
            msq_r = work.tile([1, C], F32, tag="srow")
            nc.vector.tensor_add(msq_r, s2s[:, :C], s2s[:, C:])
            nc.vector.tensor_scalar_mul(msq_r, msq_r, 1.0 / NPIX)
            var_r = work.tile([1, C], F32, tag="srow")
            nc.vector.tensor_mul(var_r, mean_r, mean_r)
            nc.vector.tensor_sub(var_r, msq_r, var_r)
            # rs = exp(-0.5*ln(var+eps))  (Ln+Exp live in one ACT table set)
            rs_r = consts.tile([1, C], F32R, tag="rsr")
            nc.scalar.activation(rs_r, var_r, AF.Ln, bias=eps1, scale=1.0)
            nc.scalar.activation(rs_r, rs_r, AF.Exp, scale=-0.5)
            murs_r = work.tile([1, C], F32R, tag="srow")
            nc.vector.tensor_mul(murs_r, mean_r, rs_r)

            # replicated [128, C] mean / rs
            mu_bc = consts.tile([P, C], F32, tag="mubc")
            rs_bc = consts.tile([P, C], F32, tag="rsbc")
            for dst, row in ((mu_bc, mean_r), (rs_bc, rs_r)):
                pb = psp.tile([P, C], F32, tag="p")
                nc.tensor.matmul(pb, rr(ones_row[:1, :]), rr(row), start=True, stop=True)
                nc.scalar.activation(dst, pb, AF.Copy)

            # M1 = mu_t - sig_t*(mu*rs) ; Srs = sig_t*rs   (rows 0..17)
            m1_sb = consts.tile([P, C], F32R, tag="m1")
            srs_sb = consts.tile([P, C], F32R, tag="srs")
            pb18 = psq.tile([P, C], F32, tag="q")
            nc.tensor.matmul(
                pb18[:LK, :], rr(ones_row[:1, :LK]), rr(murs_r), start=True, stop=True
            )
            nc.vector.tensor_mul(m1_sb[:LK, :], sig_t[:LK, :], pb18[:LK, :])
            nc.vector.tensor_sub(m1_sb[:LK, :], mu_t[:LK, :], m1_sb[:LK, :])
            pb18b = psq.tile([P, C], F32, tag="q")
            nc.tensor.matmul(
                pb18b[:LK, :], rr(ones_row[:1, :LK]), rr(rs_r), start=True, stop=True
            )
            nc.vector.tensor_mul(srs_sb[:LK, :], sig_t[:LK, :], pb18b[:LK, :])

            # ---------------- phase 2: main loop over 512-pixel tiles ----------------
            for i in range(NT):
                base = i * 4 * C  # free offset of this tile in X
                # -- channel-major xT via PE transpose (2 c-halves)
                xT_sb = work.tile([P, 2 * TW], F32R, tag="xT")
                for h in range(2):
                    trx = pstr.tile([P, TW], F32, tag="tr")
                    for m in range(4):
                        nc.tensor.transpose(
                            rr(trx[:, m * P : (m + 1) * P]),
                            rr(X[:, base + m * C + h * P : base + m * C + (h + 1) * P]),
                            rr(ident),
                        )
                    nc.scalar.activation(xT_sb[:, h * TW : (h + 1) * TW], trx, AF.Copy)

                # -- projections x_qT / x_kT [dk, 512]
                xqp = psmm.tile([P, TW], F32, tag="mm")
                for h in range(2):
                    nc.tensor.matmul(
                        xqp,
                        rr(wq_sb[:, h, :]),
                        rr(xT_sb[:, h * TW : (h + 1) * TW]),
                        start=(h == 0),
                        stop=(h == 1),
                    )
                xq_sb = work.tile([P, TW], F32R, tag="xq")
                nc.scalar.activation(xq_sb, xqp, AF.Identity, bias=bq_sb)
                xkp = psmm.tile([P, TW], F32, tag="mm")
                for h in range(2):
                    nc.tensor.matmul(
                        xkp,
                        rr(wk_sb[:, h, :]),
                        rr(xT_sb[:, h * TW : (h + 1) * TW]),
                        start=(h == 0),
                        stop=(h == 1),
                    )
                xk_sb = work.tile([P, TW], F32R, tag="xk")
                nc.scalar.activation(xk_sb, xkp, AF.Identity, bias=bk_sb)

                nc.vector.tensor_mul(xk_sb, xq_sb, xk_sb)
                prod = xk_sb

                # -- token-major cross scores + exp
                stp = psmm.tile([P, TW], F32, tag="mm")
                nc.tensor.matmul(
                    stp[:LK, :], rr(tkT_sb[:, :LK]), rr(xq_sb), start=True, stop=True
                )
                e_t = work.tile([P, TW], F32R, tag="et")
                nc.scalar.activation(e_t[:LK, :], stp[:LK, :], AF.Exp)

                # -- per 128-pixel chunk: pixel-major scores/softmax + epilogue
                for m in range(4):
                    if m % 2 == 0:
                        out_sb = outp.tile([P, 2 * C], F32, tag="out")
                    xq_c = xq_sb[:, m * P : (m + 1) * P]
                    spx = psmm.tile([P, 32], F32, tag="mm")
                    nc.tensor.matmul(
                        spx[:, :LK], rr(xq_c), rr(tkT_sb[:, :LK]), start=True, stop=True
                    )
                    nc.tensor.matmul(
                        spx[:, LK : LK + 2],
                        rr(prod[:, m * P : (m + 1) * P]),
                        rr(ones2),
                        start=True,
                        stop=True,
                    )
                    epix = work.tile([P, 32], F32, tag="epix")
                    nc.scalar.activation(epix[:, : LK + 1], spx[:, : LK + 1], AF.Exp)
                    zc = work.tile([P, 1], F32, tag="zc")
                    nc.vector.tensor_reduce(
                        zc, epix[:, : LK + 1], axis=mybir.AxisListType.X, op=OP.add
                    )
                    rz = work.tile([P, 1], F32, tag="rz")
                    nc.vector.reciprocal(rz, zc)
                    sx = work.tile([P, 1], F32, tag="sx")
                    nc.vector.tensor_mul(sx, epix[:, LK : LK + 1], rz)

                    pP = psp.tile([P, C], F32, tag="p")
                    nc.tensor.matmul(
                        pP,
                        rr(e_t[:LK, m * P : (m + 1) * P]),
                        rr(m1_sb[:LK, :]),
                        start=True,
                        stop=True,
                    )
                    pQ = psq.tile([P, C], F32, tag="q")
                    nc.tensor.matmul(
                        pQ,
                        rr(e_t[:LK, m * P : (m + 1) * P]),
                        rr(srs_sb[:LK, :]),
                        start=True,
                        stop=True,
                    )

                    x_c = X[:, base + m * C : base + (m + 1) * C]
                    d1 = ddpool.tile([P, C], F32, tag="dd")
                    nc.vector.tensor_sub(d1, x_c, mu_bc)
                    d2 = ddpool.tile([P, C], F32, tag="dd")
                    nc.vector.tensor_mul(d2, d1, rs_bc)
                    d3 = ddpool.tile([P, C], F32, tag="dd")
                    nc.vector.scalar_tensor_tensor(
                        d3, pQ, rz, x_c, op0=OP.mult, op1=OP.mult
                    )
                    d4 = ddpool.tile([P, C], F32, tag="dd")
                    nc.vector.scalar_tensor_tensor(
                        d4, pP, rz, d2, op0=OP.mult, op1=OP.add
                    )
                    d5 = ddpool.tile([P, C], F32, tag="dd")
                    nc.vector.scalar_tensor_tensor(
                        d5, x_c, sx, d3, op0=OP.mult, op1=OP.add
                    )
                    nc.vector.tensor_add(out_sb[:, (m % 2) * C : (m % 2 + 1) * C], d4, d5)
                    if m % 2 == 1:
                        nc.sync.dma_start(
                            out=out_ext[(i * 4 + m - 1) * P : (i * 4 + m + 1) * P, :].rearrange(
                                "(t p) c -> p t c", p=P
                            ),
                            in_=out_sb.rearrange("p (t c) -> p t c", c=C),
                        )

    return nc


_NC_CACHE = None


def _get_nc():
    global _NC_CACHE
    if _NC_CACHE is None:
        _NC_CACHE = _build(bacc.Bacc())
        _NC_CACHE.compile()
    return _NC_CACHE


def kernel(**inputs):
    global LAST_RESULT
    n = inputs["x"].shape[0]
    f = np.ascontiguousarray
    in_maps = []
    for b in range(n):
        in_maps.append(
            {
                "x": f(inputs["x"][b].reshape(NPIX, C).astype(np.float32)),
                "t": f(inputs["t"][b].astype(np.float32)),
                "Wq": f(inputs["Wq"].astype(np.float32)),
                "bq": f(inputs["bq"].astype(np.float32).reshape(DK, 1)),
                "Wk": f(inputs["Wk"].astype(np.float32)),
                "bk": f(inputs["bk"].astype(np.float32).reshape(DK, 1)),
                "Wtk": f(inputs["Wtk"].astype(np.float32)),
                "btk": f(inputs["btk"].astype(np.float32).reshape(1, DK)),
                "Wmu": f(inputs["Wmu"].astype(np.float32)),
                "bmu": f(inputs["bmu"].astype(np.float32).reshape(1, C)),
                "Wsig": f(inputs["Wsig"].astype(np.float32)),
                "bsig": f(inputs["bsig"].astype(np.float32).reshape(1, C)),
            }
        )
    nc = _get_nc()
    res = run_bass_kernel_spmd(
        nc, in_maps, core_ids=list(range(n)), trace=bool(os.environ.get("BASS_TRACE"))
    )
    LAST_RESULT = res
    h = w = 128
    out = np.stack([res.results[b]["out"].reshape(h, w, C) for b in range(n)])
    return out.astype(np.float32)



# revision 15
# speedup vs baseline: 1.4908x; 1.4908x over previous
"""ACAN sparse-attention kernel for 8x TRN2 NeuronCores.

Data-parallel over batch n=8 (one batch image per core, weights replicated).
Per core: x [16384, 256] instance-norm + QK self/cross scores vs 18 text
tokens + per-pixel softmax(19) + weighted mu/sigma modulation.

Math (per pixel p, channel c), with E = exp([x_q@t_k.T | x_q.x_k]) and
Z = sum_k E[k,p], rz = 1/Z:
  out = (1 + ws)*xn + wm + sx*x
      = x .* (rz * (E19.T @ srs2)) + rz * (E19.T @ m12)
  where srs2[k<18] = (sig_t+1).*rs,  srs2[18] = 1 + rs
        m12 [k<18] = mu_t - (sig_t+1).*murs,  m12[18] = -murs
        rs = 1/sqrt(var+eps), murs = mean*rs   (instance-norm stats)
Two-pass streaming: pass 1 loads x, computes scores token-major + exp once
(stored bf16) + stats partials; pass 2 re-expands E via two small PE
matmuls per 128-px chunk and a 2-op elementwise epilogue split across
DVE / Scalar / GpSimd.
"""

import os
import sys

import numpy as np

for _p in ("/opt/trn_rl_repo", "/root/.axon_site/_ro/trn_rl_repo"):
    if os.path.isdir(_p) and _p not in sys.path:
        sys.path.append(_p)

import concourse.bass as bass
import concourse.bacc as bacc
import concourse.tile as tile
from concourse import mybir
from concourse.bass_utils import run_bass_kernel_spmd

F32 = mybir.dt.float32
F32R = mybir.dt.float32r
BF16 = mybir.dt.bfloat16
AF = mybir.ActivationFunctionType
OP = mybir.AluOpType

P = 128          # partitions
C = 256          # channels
DK = 128         # query/key dim
LK = 18          # text tokens
L1 = LK + 1      # tokens + self
L33 = 33         # E rows: 0-17 cross, 18-31 zero, 32 self (ACT partition-base rule)
NPIX = 128 * 128
TW = 512         # pixels per tile
NT = NPIX // TW  # 32 tiles
EPS = 1e-5

LAST_RESULT = None


def rr(ap):
    return ap.bitcast(F32R)


def _build(nc: bass.Bass):
    x_ext = nc.declare_dram_parameter("x", [NPIX, C], F32, isOutput=False)
    t_ext = nc.declare_dram_parameter("t", [LK, C], F32, isOutput=False)
    wq_ext = nc.declare_dram_parameter("Wq", [C, DK], F32, isOutput=False)
    bq_ext = nc.declare_dram_parameter("bq", [DK, 1], F32, isOutput=False)
    wk_ext = nc.declare_dram_parameter("Wk", [C, DK], F32, isOutput=False)
    bk_ext = nc.declare_dram_parameter("bk", [DK, 1], F32, isOutput=False)
    wtk_ext = nc.declare_dram_parameter("Wtk", [C, DK], F32, isOutput=False)
    btk_ext = nc.declare_dram_parameter("btk", [1, DK], F32, isOutput=False)
    wmu_ext = nc.declare_dram_parameter("Wmu", [C, C], F32, isOutput=False)
    bmu_ext = nc.declare_dram_parameter("bmu", [1, C], F32, isOutput=False)
    wsig_ext = nc.declare_dram_parameter("Wsig", [C, C], F32, isOutput=False)
    bsig_ext = nc.declare_dram_parameter("bsig", [1, C], F32, isOutput=False)
    out_ext = nc.declare_dram_parameter("out", [NPIX, C], F32, isOutput=True)

    # block-major pixel layout: partition = pix//128, free t = pix%128
    x_blk = x_ext.rearrange("(p t) c -> p t c", p=P)
    out_blk = out_ext.rearrange("(p t) c -> p t c", p=P)

    with tile.TileContext(nc) as tc:
        with (
            tc.tile_pool(name="consts", bufs=1) as consts,
            tc.tile_pool(name="work", bufs=2) as work,
            tc.tile_pool(name="scr", bufs=1) as scr,
            tc.tile_pool(name="outp", bufs=2) as outp,
        ):
            # ---------------- constants / small inputs ----------------
            from concourse.masks import make_identity
            ident_f = consts.tile([P, P], F32, tag="identf")
            make_identity(nc, ident_f)
            ident = consts.tile([P, P], F32R, tag="ident")
            nc.scalar.activation(ident, ident_f, AF.Copy)
            ones_f = consts.tile([P, 64], F32, tag="onesf")
            nc.vector.memset(ones_f, 1.0)
            ones = consts.tile([P, 64], F32R, tag="ones")
            nc.scalar.activation(ones, ones_f, AF.Copy)
            ones_bf = consts.tile([L33, 1], BF16, tag="onesbf")
            nc.vector.memset(ones_bf, 1.0)
            eps_col = consts.tile([P, 1], F32, tag="eps")
            nc.vector.memset(eps_col, EPS)

            wq_sb = consts.tile([P, 2, DK], F32R, tag="wq")
            nc.sync.dma_start(out=wq_sb, in_=wq_ext.rearrange("(h p) d -> p h d", p=P).bitcast(F32R))
            wk_sb = consts.tile([P, 2, DK], F32R, tag="wk")
            nc.sync.dma_start(out=wk_sb, in_=wk_ext.rearrange("(h p) d -> p h d", p=P).bitcast(F32R))
            wtk_sb = consts.tile([P, 2, DK], F32R, tag="wtk")
            nc.sync.dma_start(out=wtk_sb, in_=wtk_ext.rearrange("(h p) d -> p h d", p=P).bitcast(F32R))
            wmu_sb = consts.tile([P, 2, C], F32R, tag="wmu")
            nc.sync.dma_start(out=wmu_sb, in_=wmu_ext.rearrange("(h p) d -> p h d", p=P).bitcast(F32R))
            wsig_sb = consts.tile([P, 2, C], F32R, tag="wsig")
            nc.sync.dma_start(out=wsig_sb, in_=wsig_ext.rearrange("(h p) d -> p h d", p=P).bitcast(F32R))
            bq_sb = consts.tile([P, 1], F32, tag="bq")
            nc.sync.dma_start(out=bq_sb, in_=bq_ext[:, :])
            bk_sb = consts.tile([P, 1], F32, tag="bk")
            nc.sync.dma_start(out=bk_sb, in_=bk_ext[:, :])
            btk_sb = consts.tile([1, DK], F32R, tag="btk")
            nc.sync.dma_start(out=btk_sb, in_=btk_ext[:, :].bitcast(F32R))
            bmu_sb = consts.tile([1, C], F32R, tag="bmu")
            nc.sync.dma_start(out=bmu_sb, in_=bmu_ext[:, :].bitcast(F32R))
            bsig_sb = consts.tile([1, C], F32R, tag="bsig")
            nc.sync.dma_start(out=bsig_sb, in_=bsig_ext[:, :].bitcast(F32R))
            t_sb = consts.tile([LK, C], F32R, tag="tsb")
            nc.sync.dma_start(out=t_sb, in_=t_ext[:, :].bitcast(F32R))

            # resident block-major x + bf16 exp scores for all pixels
            X = consts.tile([P, P, C], F32R, tag="X")
            E = consts.tile([L33, NPIX], BF16, tag="E")

            tT_sb = consts.tile([P, 2, 32], F32R, tag="tT")
            tkT_sb = consts.tile([P, 32], F32R, tag="tkT")
            mu_t = consts.tile([L1, C], F32, tag="mut")
            sig_t = consts.tile([L1, C], F32, tag="sigt")
            s1_st = consts.tile([P, 2, NT], F32, tag="s1st")
            s2_st = consts.tile([P, 2, NT], F32, tag="s2st")
            srs2_sb = consts.tile([L33, C], BF16, tag="srs2")
            m12_sb = consts.tile([L33, C], BF16, tag="m12")
            nc.vector.memset(E, 0.0)
            nc.vector.memset(srs2_sb, 0.0)
            nc.vector.memset(m12_sb, 0.0)

            with (
                tc.tile_pool(name="pstr", bufs=1, space="PSUM") as pstr,
                tc.tile_pool(name="psmm", bufs=1, space="PSUM") as psmm,
                tc.tile_pool(name="psc", bufs=2, space="PSUM") as psc,
            ):
                # -------- input DMA: first two tiles up front, rest rolled --------
                def dma_in(i):
                    nc.sync.dma_start(
                        out=X[:, 4 * i : 4 * i + 4, :],
                        in_=x_blk[:, 4 * i : 4 * i + 4, :].bitcast(F32R),
                    )

                dma_in(0)
                dma_in(1)

                # ---------------- phase 0: text projections ----------------
                trT = pstr.tile([P, TW], F32, tag="trh0")
                for h in range(2):
                    nc.tensor.transpose(
                        rr(trT[:, h * 32 : h * 32 + LK]),
                        t_sb[:, h * P : (h + 1) * P],
                        ident[:LK, :LK],
                    )
                nc.scalar.activation(tT_sb[:, 0, :LK], trT[:, 0:LK], AF.Copy)
                nc.scalar.activation(tT_sb[:, 1, :LK], trT[:, 32 : 32 + LK], AF.Copy)

                # t_kT [dk, 18] = Wtk.T @ tT + btk
                tkp = psmm.tile([P, 2 * TW], F32, tag="qk")
                for h in range(2):
                    nc.tensor.matmul(
                        tkp[:, :LK],
                        wtk_sb[:, h, :],
                        tT_sb[:, h, :LK],
                        start=(h == 0),
                        stop=False,
                    )
                nc.tensor.matmul(
                    tkp[:, :LK], btk_sb, ones[:1, :LK], start=False, stop=True
                )
                nc.scalar.activation(tkT_sb[:, :LK], tkp[:, :LK], AF.Copy)

                # mu_t / sigma_t [18, C]
                for dst, wsb, brow in ((mu_t, wmu_sb, bmu_sb), (sig_t, wsig_sb, bsig_sb)):
                    pp = psc.tile([P, TW], F32, tag="sc")
                    for h in range(2):
                        nc.tensor.matmul(
                            pp[:LK, :C],
                            tT_sb[:, h, :LK],
                            wsb[:, h, :],
                            start=(h == 0),
                            stop=False,
                        )
                    nc.tensor.matmul(
                        pp[:LK, :C], ones[:1, :LK], brow, start=False, stop=True
                    )
                    nc.scalar.activation(dst[:LK, :], pp[:LK, :C], AF.Copy)

                # ---------------- pass 1: stream tiles ----------------
                for i in range(NT):
                    if i + 2 < NT:
                        dma_in(i + 2)

                    # channel-major xT via PE transpose (2 c-halves)
                    trx0 = pstr.tile([P, TW], F32, tag="trh0")
                    trx1 = pstr.tile([P, TW], F32, tag="trh1")
                    for m in range(4):
                        nc.tensor.transpose(
                            rr(trx0[:, m * P : (m + 1) * P]),
                            X[:, 4 * i + m, 0:P],
                            ident,
                        )
                        nc.tensor.transpose(
                            rr(trx1[:, m * P : (m + 1) * P]),
                            X[:, 4 * i + m, P:C],
                            ident,
                        )
                    xT0 = work.tile([P, TW], F32R, tag="xT0")
                    nc.scalar.activation(
                        xT0, trx0, AF.Copy, accum_out=s1_st[:, 0, i : i + 1]
                    )
                    xT1 = work.tile([P, TW], F32R, tag="xT1")
                    nc.scalar.activation(
                        xT1, trx1, AF.Copy, accum_out=s1_st[:, 1, i : i + 1]
                    )

                    # sum of squares partials on GpSimd (SBUF only)
                    sq0 = scr.tile([P, TW], F32, tag="sq")
                    nc.vector.scalar_tensor_tensor(
                        sq0, xT0, 1.0, xT0, op0=OP.mult, op1=OP.mult,
                        accum_out=s2_st[:, 0, i : i + 1],
                    )
                    sq1 = scr.tile([P, TW], F32, tag="sq")
                    nc.scalar.activation(
                        sq1, xT1, AF.Square, accum_out=s2_st[:, 1, i : i + 1]
                    )

                    # projections x_qT / x_kT [dk, 512] (one psum tile)
                    pqk = psmm.tile([P, 2 * TW], F32, tag="qk")
                    for h in range(2):
                        nc.tensor.matmul(
                            pqk[:, :TW], wq_sb[:, h, :], xT0 if h == 0 else xT1,
                            start=(h == 0), stop=(h == 1),
                        )
                    for h in range(2):
                        nc.tensor.matmul(
                            pqk[:, TW:], wk_sb[:, h, :], xT0 if h == 0 else xT1,
                            start=(h == 0), stop=(h == 1),
                        )
                    xq_sb = work.tile([P, TW], F32R, tag="xq")
                    nc.vector.tensor_scalar_add(xq_sb, pqk[:, :TW], bq_sb[:, :])
                    prod = work.tile([P, TW], F32R, tag="prod")
                    nc.vector.scalar_tensor_tensor(
                        prod, pqk[:, TW:], bk_sb[:, :], xq_sb, op0=OP.add, op1=OP.mult
                    )

                    # token-major scores: cross at rows 0:18, self at row 32
                    # (matmul output base partition must be 0/32/64)
                    sc19 = psc.tile([P, TW], F32, tag="sc")
                    nc.tensor.matmul(
                        sc19[:LK, :], tkT_sb[:, :LK], xq_sb, start=True, stop=True
                    )
                    sc_self = psc.tile([1, TW], F32, tag="self")
                    nc.tensor.matmul(
                        sc_self, ones[:, 0:1], prod, start=True, stop=True
                    )
                    nc.scalar.activation(
                        E[:LK, i * TW : (i + 1) * TW], sc19[:LK, :], AF.Exp
                    )
                    nc.scalar.activation(
                        E[32:33, i * TW : (i + 1) * TW], sc_self, AF.Exp
                    )

            # ---------------- stats finalize ----------------
            with tc.tile_pool(name="psfin", bufs=1, space="PSUM") as psfin:
                s1t = scr.tile([P, 2], F32, tag="s1t")
                nc.vector.tensor_reduce(s1t, s1_st, axis=mybir.AxisListType.X, op=OP.add)
                s2t = scr.tile([P, 2], F32, tag="s2t")
                nc.vector.tensor_reduce(s2t, s2_st, axis=mybir.AxisListType.X, op=OP.add)
                mean2 = scr.tile([P, 2], F32, tag="mean2")
                nc.vector.tensor_scalar_mul(mean2, s1t, 1.0 / NPIX)
                var2 = scr.tile([P, 2], F32, tag="var2")
                nc.vector.tensor_mul(var2, mean2, mean2)
                nc.vector.scalar_tensor_tensor(
                    var2, s2t, 1.0 / NPIX, var2, op0=OP.mult, op1=OP.subtract
                )
                # quad cols: rs_h0, rs_h1, murs_h0, murs_h1
                quad = scr.tile([P, 4], F32R, tag="quad")
                nc.scalar.activation(quad[:, 0:2], var2, AF.Ln, bias=eps_col)
                nc.scalar.activation(quad[:, 0:2], quad[:, 0:2], AF.Exp, scale=-0.5)
                nc.vector.tensor_mul(quad[:, 2:4], mean2, quad[:, 0:2])

                pq4 = psfin.tile([1, 4, P], F32, tag="q4")
                for m in range(4):
                    nc.tensor.transpose(rr(pq4[:, m, :]), quad[:, m : m + 1], ident)
                rows4 = scr.tile([1, 2, C], F32R, tag="rows4")
                nc.scalar.activation(rows4[:, 0, 0:P], pq4[:, 0, :], AF.Copy)
                nc.scalar.activation(rows4[:, 0, P:C], pq4[:, 1, :], AF.Copy)
                nc.scalar.activation(rows4[:, 1, 0:P], pq4[:, 2, :], AF.Copy)
                nc.scalar.activation(rows4[:, 1, P:C], pq4[:, 3, :], AF.Copy)

                # srs2 = [(sig_t+1).*rs ; 1+rs],  m12 = [mu_t-(sig_t+1).*murs ; -murs]
                p_rs = psfin.tile([L33, C], F32, tag="rsrep")
                nc.tensor.matmul(p_rs, ones[:1, :L33], rows4[:, 0, :], start=True, stop=True)
                p_mu = psfin.tile([L33, C], F32, tag="mursrep")
                nc.tensor.matmul(p_mu, ones[:1, :L33], rows4[:, 1, :], start=True, stop=True)

                nc.vector.scalar_tensor_tensor(
                    srs2_sb[:LK, :], sig_t[:LK, :], 1.0, p_rs[:LK, :],
                    op0=OP.add, op1=OP.mult,
                )
                nc.vector.tensor_scalar_add(srs2_sb[32:33, :], p_rs[32:33, :], 1.0)
                tmp18 = scr.tile([L1, C], F32, tag="tmp18")
                nc.vector.scalar_tensor_tensor(
                    tmp18[:LK, :], sig_t[:LK, :], 1.0, p_mu[:LK, :],
                    op0=OP.add, op1=OP.mult,
                )
                nc.vector.tensor_sub(m12_sb[:LK, :], mu_t[:LK, :], tmp18[:LK, :])
                nc.vector.tensor_scalar_mul(m12_sb[32:33, :], p_mu[32:33, :], -1.0)

            # ---------------- pass 2: P/Q matmuls + epilogue ----------------
            with (
                tc.tile_pool(name="psq", bufs=3, space="PSUM") as psq,
                tc.tile_pool(name="psp", bufs=3, space="PSUM") as psp,
                tc.tile_pool(name="psz", bufs=2, space="PSUM") as psz,
            ):
                for i in range(NT):
                    pz = psz.tile([P, 4], F32, tag="z")
                    pqs = []
                    pps = []
                    for m in range(4):
                        ec = E[:, (4 * i + m) * P : (4 * i + m + 1) * P]
                        pQ = psq.tile([P, C], F32, tag="q")
                        nc.tensor.matmul(pQ, ec, srs2_sb, start=True, stop=True)
                        pP = psp.tile([P, C], F32, tag="p")
                        nc.tensor.matmul(pP, ec, m12_sb, start=True, stop=True)
                        nc.tensor.matmul(
                            pz[:, m : m + 1], ec, ones_bf,
                            start=True, stop=True,
                        )
                        pqs.append(pQ)
                        pps.append(pP)
                    rz = work.tile([P, 4], F32, tag="rz")
                    nc.vector.reciprocal(rz, pz)
                    for m in range(4):
                        if m % 2 == 0:
                            out_sb = outp.tile([P, 2, C], F32, tag="out")
                        v1 = work.tile([P, C], F32, tag="v1")
                        nc.vector.scalar_tensor_tensor(
                            v1, pqs[m], rz[:, m : m + 1], X[:, 4 * i + m, :].bitcast(F32),
                            op0=OP.mult, op1=OP.mult,
                        )
                        t2 = work.tile([P, C], F32, tag="t2")
                        nc.scalar.activation(
                            t2, pps[m], AF.Copy, scale=rz[:, m : m + 1]
                        )
                        nc.gpsimd.tensor_add(out_sb[:, m % 2, :], v1, t2)
                        if m % 2 == 1:
                            nc.sync.dma_start(
                                out=out_blk[:, 4 * i + m - 1 : 4 * i + m + 1, :],
                                in_=out_sb,
                            )

    return nc


_NC_CACHE = None


def _get_nc():
    global _NC_CACHE
    if _NC_CACHE is None:
        _NC_CACHE = _build(bacc.Bacc())
        _NC_CACHE.compile()
    return _NC_CACHE


def kernel(**inputs):
    global LAST_RESULT
    n = inputs["x"].shape[0]
    f = np.ascontiguousarray
    in_maps = []
    for b in range(n):
        in_maps.append(
            {
                "x": f(inputs["x"][b].reshape(NPIX, C).astype(np.float32)),
                "t": f(inputs["t"][b].astype(np.float32)),
                "Wq": f(inputs["Wq"].astype(np.float32)),
                "bq": f(inputs["bq"].astype(np.float32).reshape(DK, 1)),
                "Wk": f(inputs["Wk"].astype(np.float32)),
                "bk": f(inputs["bk"].astype(np.float32).reshape(DK, 1)),
                "Wtk": f(inputs["Wtk"].astype(np.float32)),
                "btk": f(inputs["btk"].astype(np.float32).reshape(1, DK)),
                "Wmu": f(inputs["Wmu"].astype(np.float32)),
                "bmu": f(inputs["bmu"].astype(np.float32).reshape(1, C)),
                "Wsig": f(inputs["Wsig"].astype(np.float32)),
                "bsig": f(inputs["bsig"].astype(np.float32).reshape(1, C)),
            }
        )
    nc = _get_nc()
    res = run_bass_kernel_spmd(
        nc, in_maps, core_ids=list(range(n)), trace=bool(os.environ.get("BASS_TRACE"))
    )
    LAST_RESULT = res
    h = w = 128
    out = np.stack([res.results[b]["out"].reshape(h, w, C) for b in range(n)])
    return out.astype(np.float32)


# revision 20
# speedup vs baseline: 1.4938x; 1.0020x over previous
"""ACAN sparse-attention kernel for 8x TRN2 NeuronCores.

Data-parallel over batch n=8 (one batch image per core, weights replicated).
Per core: x [16384, 256] instance-norm + QK self/cross scores vs 18 text
tokens + per-pixel softmax(19) + weighted mu/sigma modulation.

Math (per pixel p, channel c), with E = exp([x_q@t_k.T | x_q.x_k]) and
Z = sum_k E[k,p], rz = 1/Z:
  out = (1 + ws)*xn + wm + sx*x
      = x .* (rz * (E19.T @ srs2)) + rz * (E19.T @ m12)
  where srs2[k<18] = (sig_t+1).*rs,  srs2[18] = 1 + rs
        m12 [k<18] = mu_t - (sig_t+1).*murs,  m12[18] = -murs
        rs = 1/sqrt(var+eps), murs = mean*rs   (instance-norm stats)
Two-pass streaming: pass 1 loads x, computes scores token-major + exp once
(stored bf16) + stats partials; pass 2 re-expands E via two small PE
matmuls per 128-px chunk and a 2-op elementwise epilogue split across
DVE / Scalar / GpSimd.
"""

import os
import sys

import numpy as np

for _p in ("/opt/trn_rl_repo", "/root/.axon_site/_ro/trn_rl_repo"):
    if os.path.isdir(_p) and _p not in sys.path:
        sys.path.append(_p)

import concourse.bass as bass
import concourse.bacc as bacc
import concourse.tile as tile
from concourse import mybir
from concourse.bass_utils import run_bass_kernel_spmd

F32 = mybir.dt.float32
F32R = mybir.dt.float32r
BF16 = mybir.dt.bfloat16
AF = mybir.ActivationFunctionType
OP = mybir.AluOpType

P = 128          # partitions
C = 256          # channels
DK = 128         # query/key dim
LK = 18          # text tokens
L1 = LK + 1      # tokens + self
L33 = 33         # E rows: 0-17 cross, 18-31 zero, 32 self (ACT partition-base rule)
NPIX = 128 * 128
TW = 512         # pixels per tile
NT = NPIX // TW  # 32 tiles
EPS = 1e-5

LAST_RESULT = None


def rr(ap):
    return ap.bitcast(F32R)


def _build(nc: bass.Bass):
    x_ext = nc.declare_dram_parameter("x", [NPIX, C], F32, isOutput=False)
    t_ext = nc.declare_dram_parameter("t", [LK, C], F32, isOutput=False)
    wq_ext = nc.declare_dram_parameter("Wq", [C, DK], F32, isOutput=False)
    bq_ext = nc.declare_dram_parameter("bq", [DK, 1], F32, isOutput=False)
    wk_ext = nc.declare_dram_parameter("Wk", [C, DK], F32, isOutput=False)
    bk_ext = nc.declare_dram_parameter("bk", [DK, 1], F32, isOutput=False)
    wtk_ext = nc.declare_dram_parameter("Wtk", [C, DK], F32, isOutput=False)
    btk_ext = nc.declare_dram_parameter("btk", [1, DK], F32, isOutput=False)
    wmu_ext = nc.declare_dram_parameter("Wmu", [C, C], F32, isOutput=False)
    bmu_ext = nc.declare_dram_parameter("bmu", [1, C], F32, isOutput=False)
    wsig_ext = nc.declare_dram_parameter("Wsig", [C, C], F32, isOutput=False)
    bsig_ext = nc.declare_dram_parameter("bsig", [1, C], F32, isOutput=False)
    out_ext = nc.declare_dram_parameter("out", [NPIX, C], F32, isOutput=True)

    # block-major pixel layout: partition = pix//128, free t = pix%128
    x_blk = x_ext.rearrange("(p t) c -> p t c", p=P)
    out_blk = out_ext.rearrange("(p t) c -> p t c", p=P)

    with tile.TileContext(nc) as tc:
        with (
            tc.tile_pool(name="consts", bufs=1) as consts,
            tc.tile_pool(name="work", bufs=2) as work,
            tc.tile_pool(name="scr", bufs=1) as scr,
            tc.tile_pool(name="outp", bufs=2) as outp,
        ):
            # ---------------- constants / small inputs ----------------
            from concourse.masks import make_identity
            ident_f = consts.tile([P, P], F32, tag="identf")
            make_identity(nc, ident_f)
            ident = consts.tile([P, P], F32R, tag="ident")
            nc.scalar.activation(ident, ident_f, AF.Copy)
            ones_f = consts.tile([P, 64], F32, tag="onesf")
            nc.vector.memset(ones_f, 1.0)
            ones = consts.tile([P, 64], F32R, tag="ones")
            nc.scalar.activation(ones, ones_f, AF.Copy)
            ones_bf = consts.tile([L33, 1], BF16, tag="onesbf")
            nc.vector.memset(ones_bf, 1.0)
            eps_col = consts.tile([P, 1], F32, tag="eps")
            nc.vector.memset(eps_col, EPS)

            wq_sb = consts.tile([P, 2, DK], F32R, tag="wq")
            nc.sync.dma_start(out=wq_sb, in_=wq_ext.rearrange("(h p) d -> p h d", p=P).bitcast(F32R))
            wk_sb = consts.tile([P, 2, DK], F32R, tag="wk")
            nc.sync.dma_start(out=wk_sb, in_=wk_ext.rearrange("(h p) d -> p h d", p=P).bitcast(F32R))
            wtk_sb = consts.tile([P, 2, DK], F32R, tag="wtk")
            nc.sync.dma_start(out=wtk_sb, in_=wtk_ext.rearrange("(h p) d -> p h d", p=P).bitcast(F32R))

            bq_sb = consts.tile([P, 1], F32, tag="bq")
            nc.sync.dma_start(out=bq_sb, in_=bq_ext[:, :])
            bk_sb = consts.tile([P, 1], F32, tag="bk")
            nc.sync.dma_start(out=bk_sb, in_=bk_ext[:, :])
            btk_sb = consts.tile([1, DK], F32R, tag="btk")
            nc.sync.dma_start(out=btk_sb, in_=btk_ext[:, :].bitcast(F32R))
            bmu_sb = consts.tile([1, C], F32R, tag="bmu")
            nc.sync.dma_start(out=bmu_sb, in_=bmu_ext[:, :].bitcast(F32R))
            bsig_sb = consts.tile([1, C], F32R, tag="bsig")
            nc.sync.dma_start(out=bsig_sb, in_=bsig_ext[:, :].bitcast(F32R))
            t_sb = consts.tile([LK, C], F32R, tag="tsb")
            nc.sync.dma_start(out=t_sb, in_=t_ext[:, :].bitcast(F32R))

            # resident block-major x + bf16 exp scores for all pixels
            X = consts.tile([P, P, C], F32R, tag="X")
            E = consts.tile([L33, NPIX], BF16, tag="E")
            # Wmu/Wsig borrow the X tail; consumed in phase 0 before x js 112+
            # arrive (dma_in(7) is issued at i=24)
            wmu_sb = X[:, 112:114, :]
            nc.sync.dma_start(out=wmu_sb, in_=wmu_ext.rearrange("(h p) d -> p h d", p=P).bitcast(F32R))
            wsig_sb = X[:, 114:116, :]
            nc.sync.dma_start(out=wsig_sb, in_=wsig_ext.rearrange("(h p) d -> p h d", p=P).bitcast(F32R))

            tT_sb = consts.tile([P, 2, 32], F32R, tag="tT")
            tkT_sb = consts.tile([P, 32], F32R, tag="tkT")
            mu_t = consts.tile([L1, C], F32, tag="mut")
            sig_t = consts.tile([L1, C], F32, tag="sigt")
            bn_st = consts.tile([P, 2, NT, 6], F32, tag="bnst")
            srs2_sb = consts.tile([L33, C], BF16, tag="srs2")
            m12_sb = consts.tile([L33, C], BF16, tag="m12")
            nc.vector.memset(E, 0.0)
            nc.vector.memset(srs2_sb, 0.0)
            nc.vector.memset(m12_sb, 0.0)

            with (
                tc.tile_pool(name="pstr", bufs=1, space="PSUM") as pstr,
                tc.tile_pool(name="psmm", bufs=1, space="PSUM") as psmm,
                tc.tile_pool(name="psc", bufs=2, space="PSUM") as psc,
            ):
                # -------- input DMA: first two tiles up front, rest rolled --------
                def dma_in(g):
                    nc.sync.dma_start(
                        out=X[:, 16 * g : 16 * g + 16, :],
                        in_=x_blk[:, 16 * g : 16 * g + 16, :].bitcast(F32R),
                    )

                dma_in(0)

                # ---------------- phase 0: text projections ----------------
                trT = pstr.tile([P, TW], F32, tag="trh0")
                for h in range(2):
                    nc.tensor.transpose(
                        rr(trT[:, h * 32 : h * 32 + LK]),
                        t_sb[:, h * P : (h + 1) * P],
                        ident[:LK, :LK],
                    )
                nc.scalar.activation(tT_sb[:, 0, :LK], trT[:, 0:LK], AF.Copy)
                nc.scalar.activation(tT_sb[:, 1, :LK], trT[:, 32 : 32 + LK], AF.Copy)

                # t_kT [dk, 18] = Wtk.T @ tT + btk
                tkp = psmm.tile([P, 2 * TW], F32, tag="qk")
                for h in range(2):
                    nc.tensor.matmul(
                        tkp[:, :LK],
                        wtk_sb[:, h, :],
                        tT_sb[:, h, :LK],
                        start=(h == 0),
                        stop=False,
                    )
                nc.tensor.matmul(
                    tkp[:, :LK], btk_sb, ones[:1, :LK], start=False, stop=True
                )
                nc.scalar.activation(tkT_sb[:, :LK], tkp[:, :LK], AF.Copy)

                # mu_t / sigma_t [18, C]
                for dst, wsb, brow in ((mu_t, wmu_sb, bmu_sb), (sig_t, wsig_sb, bsig_sb)):
                    pp = psc.tile([P, TW], F32, tag="sc")
                    for h in range(2):
                        nc.tensor.matmul(
                            pp[:LK, :C],
                            tT_sb[:, h, :LK],
                            wsb[:, h, :],
                            start=(h == 0),
                            stop=False,
                        )
                    nc.tensor.matmul(
                        pp[:LK, :C], ones[:1, :LK], brow, start=False, stop=True
                    )
                    nc.scalar.activation(dst[:LK, :], pp[:LK, :C], AF.Copy)

                # ---------------- pass 1: stream tiles ----------------
                for i in range(NT):
                    if i % 4 == 0 and i // 4 + 1 < 8:
                        dma_in(i // 4 + 1)

                    # channel-major xT via PE transpose (2 c-halves)
                    trx0 = pstr.tile([P, TW], F32, tag="trh0")
                    trx1 = pstr.tile([P, TW], F32, tag="trh1")
                    for m in range(4):
                        nc.tensor.transpose(
                            rr(trx0[:, m * P : (m + 1) * P]),
                            X[:, 4 * i + m, 0:P],
                            ident,
                        )
                        nc.tensor.transpose(
                            rr(trx1[:, m * P : (m + 1) * P]),
                            X[:, 4 * i + m, P:C],
                            ident,
                        )
                    xT0 = work.tile([P, TW], F32R, tag="xT0")
                    nc.scalar.activation(xT0, trx0, AF.Copy)
                    xT1 = work.tile([P, TW], F32R, tag="xT1")
                    nc.scalar.activation(xT1, trx1, AF.Copy)

                    # instance-norm stats: fused mean/var per half on DVE
                    nc.vector.bn_stats(bn_st[:, 0, i, :], xT0)
                    nc.vector.bn_stats(bn_st[:, 1, i, :], xT1)

                    # projections x_qT / x_kT [dk, 512] (one psum tile)
                    pqk = psmm.tile([P, 2 * TW], F32, tag="qk")
                    for h in range(2):
                        nc.tensor.matmul(
                            pqk[:, :TW], wq_sb[:, h, :], xT0 if h == 0 else xT1,
                            start=(h == 0), stop=(h == 1),
                        )
                    for h in range(2):
                        nc.tensor.matmul(
                            pqk[:, TW:], wk_sb[:, h, :], xT0 if h == 0 else xT1,
                            start=(h == 0), stop=(h == 1),
                        )
                    xq_sb = work.tile([P, TW], F32R, tag="xq")
                    nc.vector.tensor_scalar_add(xq_sb, pqk[:, :TW], bq_sb[:, :])
                    xk_sb = work.tile([P, TW], F32R, tag="xk")
                    nc.scalar.activation(xk_sb, pqk[:, TW:], AF.Identity, bias=bk_sb)
                    prod = work.tile([P, TW], F32R, tag="prod")
                    nc.gpsimd.tensor_mul(prod, xq_sb, xk_sb)

                    # token-major scores: cross at rows 0:18, self at row 32
                    # (matmul output base partition must be 0/32/64)
                    sc19 = psc.tile([P, TW], F32, tag="sc")
                    nc.tensor.matmul(
                        sc19[:LK, :], tkT_sb[:, :LK], xq_sb, start=True, stop=True
                    )
                    sc_self = psc.tile([1, TW], F32, tag="self")
                    nc.tensor.matmul(
                        sc_self, ones[:, 0:1], prod, start=True, stop=True
                    )
                    nc.scalar.activation(
                        E[:LK, i * TW : (i + 1) * TW], sc19[:LK, :], AF.Exp
                    )
                    nc.scalar.activation(
                        E[32:33, i * TW : (i + 1) * TW], sc_self, AF.Exp
                    )

            # ---------------- stats finalize ----------------
            with tc.tile_pool(name="psfin", bufs=1, space="PSUM") as psfin:
                agg = scr.tile([P, 2, 2], F32, tag="agg")
                nc.vector.bn_aggr(agg[:, 0, :], bn_st[:, 0, :, :])
                nc.vector.bn_aggr(agg[:, 1, :], bn_st[:, 1, :, :])
                mean2 = agg[:, :, 0]
                var2 = agg[:, :, 1]
                # quad cols: rs_h0, rs_h1, murs_h0, murs_h1
                quad = scr.tile([P, 4], F32R, tag="quad")
                nc.scalar.activation(quad[:, 0:2], var2, AF.Ln, bias=eps_col)
                nc.scalar.activation(quad[:, 0:2], quad[:, 0:2], AF.Exp, scale=-0.5)
                nc.vector.tensor_mul(quad[:, 2:4], mean2, quad[:, 0:2])

                pq4 = psfin.tile([1, 4, P], F32, tag="q4")
                for m in range(4):
                    nc.tensor.transpose(rr(pq4[:, m, :]), quad[:, m : m + 1], ident)
                rows4 = scr.tile([1, 2, C], F32R, tag="rows4")
                nc.scalar.activation(rows4[:, 0, 0:P], pq4[:, 0, :], AF.Copy)
                nc.scalar.activation(rows4[:, 0, P:C], pq4[:, 1, :], AF.Copy)
                nc.scalar.activation(rows4[:, 1, 0:P], pq4[:, 2, :], AF.Copy)
                nc.scalar.activation(rows4[:, 1, P:C], pq4[:, 3, :], AF.Copy)

                # srs2 = [(sig_t+1).*rs ; 1+rs],  m12 = [mu_t-(sig_t+1).*murs ; -murs]
                p_rs = psfin.tile([L33, C], F32, tag="rsrep")
                nc.tensor.matmul(p_rs, ones[:1, :L33], rows4[:, 0, :], start=True, stop=True)
                p_mu = psfin.tile([L33, C], F32, tag="mursrep")
                nc.tensor.matmul(p_mu, ones[:1, :L33], rows4[:, 1, :], start=True, stop=True)

                nc.vector.scalar_tensor_tensor(
                    srs2_sb[:LK, :], sig_t[:LK, :], 1.0, p_rs[:LK, :],
                    op0=OP.add, op1=OP.mult,
                )
                nc.vector.tensor_scalar_add(srs2_sb[32:33, :], p_rs[32:33, :], 1.0)
                tmp18 = scr.tile([L1, C], F32, tag="tmp18")
                nc.vector.scalar_tensor_tensor(
                    tmp18[:LK, :], sig_t[:LK, :], 1.0, p_mu[:LK, :],
                    op0=OP.add, op1=OP.mult,
                )
                nc.vector.tensor_sub(m12_sb[:LK, :], mu_t[:LK, :], tmp18[:LK, :])
                nc.vector.tensor_scalar_mul(m12_sb[32:33, :], p_mu[32:33, :], -1.0)

            # ---------------- pass 2: P/Q matmuls + epilogue ----------------
            with (
                tc.tile_pool(name="psq", bufs=4, space="PSUM") as psq,
                tc.tile_pool(name="psz", bufs=1, space="PSUM") as psz,
            ):
                for i in range(NT):
                    pz = psz.tile([P, 4], F32, tag="z")
                    pqps = []
                    for m in range(4):
                        ec = E[:, (4 * i + m) * P : (4 * i + m + 1) * P]
                        pqp = psq.tile([P, 2, C], F32, tag="qp")
                        nc.tensor.matmul(pqp[:, 0, :], ec, srs2_sb, start=True, stop=True)
                        nc.tensor.matmul(pqp[:, 1, :], ec, m12_sb, start=True, stop=True)
                        nc.tensor.matmul(
                            pz[:, m : m + 1], ec, ones_bf,
                            start=True, stop=True,
                        )
                        pqps.append(pqp)
                    rz = work.tile([P, 4], F32, tag="rz")
                    nc.vector.reciprocal(rz, pz)
                    for m in range(4):
                        if m % 2 == 0:
                            out_sb = outp.tile([P, 2, C], F32, tag="out")
                        v1 = work.tile([P, C], F32, tag="v1")
                        nc.vector.scalar_tensor_tensor(
                            v1, pqps[m][:, 0, :], rz[:, m : m + 1],
                            X[:, 4 * i + m, :].bitcast(F32),
                            op0=OP.mult, op1=OP.mult,
                        )
                        t2 = work.tile([P, C], F32, tag="t2")
                        nc.scalar.activation(
                            t2, pqps[m][:, 1, :], AF.Copy, scale=rz[:, m : m + 1]
                        )
                        nc.gpsimd.tensor_add(out_sb[:, m % 2, :], v1, t2)
                        if m % 2 == 1:
                            nc.sync.dma_start(
                                out=out_blk[:, 4 * i + m - 1 : 4 * i + m + 1, :],
                                in_=out_sb,
                            )

    return nc


_NC_CACHE = None


def _get_nc():
    global _NC_CACHE
    if _NC_CACHE is None:
        _NC_CACHE = _build(bacc.Bacc())
        _NC_CACHE.compile()
    return _NC_CACHE


def kernel(**inputs):
    global LAST_RESULT
    n = inputs["x"].shape[0]
    f = np.ascontiguousarray
    in_maps = []
    for b in range(n):
        in_maps.append(
            {
                "x": f(inputs["x"][b].reshape(NPIX, C).astype(np.float32)),
                "t": f(inputs["t"][b].astype(np.float32)),
                "Wq": f(inputs["Wq"].astype(np.float32)),
                "bq": f(inputs["bq"].astype(np.float32).reshape(DK, 1)),
                "Wk": f(inputs["Wk"].astype(np.float32)),
                "bk": f(inputs["bk"].astype(np.float32).reshape(DK, 1)),
                "Wtk": f(inputs["Wtk"].astype(np.float32)),
                "btk": f(inputs["btk"].astype(np.float32).reshape(1, DK)),
                "Wmu": f(inputs["Wmu"].astype(np.float32)),
                "bmu": f(inputs["bmu"].astype(np.float32).reshape(1, C)),
                "Wsig": f(inputs["Wsig"].astype(np.float32)),
                "bsig": f(inputs["bsig"].astype(np.float32).reshape(1, C)),
            }
        )
    nc = _get_nc()
    res = run_bass_kernel_spmd(
        nc, in_maps, core_ids=list(range(n)), trace=bool(os.environ.get("BASS_TRACE"))
    )
    LAST_RESULT = res
    h = w = 128
    out = np.stack([res.results[b]["out"].reshape(h, w, C) for b in range(n)])
    return out.astype(np.float32)


# revision 21
# speedup vs baseline: 1.6019x; 1.0724x over previous
"""ACAN sparse-attention kernel for 8x TRN2 NeuronCores.

Data-parallel over batch n=8 (one batch image per core, weights replicated).
Per core: x [16384, 256] instance-norm + QK self/cross scores vs 18 text
tokens + per-pixel softmax(19) + weighted mu/sigma modulation.

Math (per pixel p, channel c), with E = exp([x_q@t_k.T | x_q.x_k]) and
Z = sum_k E[k,p], rz = 1/Z:
  out = (1 + ws)*xn + wm + sx*x
      = x .* (rz * (E19.T @ srs2)) + rz * (E19.T @ m12)
  where srs2[k<18] = (sig_t+1).*rs,  srs2[18] = 1 + rs
        m12 [k<18] = mu_t - (sig_t+1).*murs,  m12[18] = -murs
        rs = 1/sqrt(var+eps), murs = mean*rs   (instance-norm stats)
Two-pass streaming: pass 1 loads x, computes scores token-major + exp once
(stored bf16) + stats partials; pass 2 re-expands E via two small PE
matmuls per 128-px chunk and a 2-op elementwise epilogue split across
DVE / Scalar / GpSimd.
"""

import os
import sys

import numpy as np

for _p in ("/opt/trn_rl_repo", "/root/.axon_site/_ro/trn_rl_repo"):
    if os.path.isdir(_p) and _p not in sys.path:
        sys.path.append(_p)

import concourse.bass as bass
import concourse.bacc as bacc
import concourse.tile as tile
from concourse import mybir
from concourse.bass_utils import run_bass_kernel_spmd

F32 = mybir.dt.float32
F32R = mybir.dt.float32r
BF16 = mybir.dt.bfloat16
AF = mybir.ActivationFunctionType
OP = mybir.AluOpType

P = 128          # partitions
C = 256          # channels
DK = 128         # query/key dim
LK = 18          # text tokens
L1 = LK + 1      # tokens + self
L33 = 33         # E rows: 0-17 cross, 18-31 zero, 32 self (ACT partition-base rule)
NPIX = 128 * 128
TW = 512         # pixels per tile
NT = NPIX // TW  # 32 tiles
EPS = 1e-5

LAST_RESULT = None


def rr(ap):
    return ap.bitcast(F32R)


def _build(nc: bass.Bass):
    x_ext = nc.declare_dram_parameter("x", [NPIX, C], F32, isOutput=False)
    t_ext = nc.declare_dram_parameter("t", [LK, C], F32, isOutput=False)
    wq_ext = nc.declare_dram_parameter("Wq", [C, DK], F32, isOutput=False)
    bq_ext = nc.declare_dram_parameter("bq", [DK, 1], F32, isOutput=False)
    wk_ext = nc.declare_dram_parameter("Wk", [C, DK], F32, isOutput=False)
    bk_ext = nc.declare_dram_parameter("bk", [DK, 1], F32, isOutput=False)
    wtk_ext = nc.declare_dram_parameter("Wtk", [C, DK], F32, isOutput=False)
    btk_ext = nc.declare_dram_parameter("btk", [1, DK], F32, isOutput=False)
    wmu_ext = nc.declare_dram_parameter("Wmu", [C, C], F32, isOutput=False)
    bmu_ext = nc.declare_dram_parameter("bmu", [1, C], F32, isOutput=False)
    wsig_ext = nc.declare_dram_parameter("Wsig", [C, C], F32, isOutput=False)
    bsig_ext = nc.declare_dram_parameter("bsig", [1, C], F32, isOutput=False)
    out_ext = nc.declare_dram_parameter("out", [NPIX, C], F32, isOutput=True)

    # block-major pixel layout: partition = pix//128, free t = pix%128
    x_blk = x_ext.rearrange("(p t) c -> p t c", p=P)
    out_blk = out_ext.rearrange("(p t) c -> p t c", p=P)

    with tile.TileContext(nc) as tc:
        with (
            tc.tile_pool(name="consts", bufs=1) as consts,
            tc.tile_pool(name="work", bufs=2) as work,
            tc.tile_pool(name="scr", bufs=1) as scr,
            tc.tile_pool(name="outp", bufs=2) as outp,
        ):
            # ---------------- constants / small inputs ----------------
            from concourse.masks import make_identity
            ident_f = consts.tile([P, P], F32, tag="identf")
            make_identity(nc, ident_f)
            ident = consts.tile([P, P], F32R, tag="ident")
            nc.scalar.activation(ident, ident_f, AF.Copy)
            ones_f = consts.tile([P, 64], F32, tag="onesf")
            nc.vector.memset(ones_f, 1.0)
            ones = consts.tile([P, 64], F32R, tag="ones")
            nc.scalar.activation(ones, ones_f, AF.Copy)
            ones_bf = consts.tile([L33, 1], BF16, tag="onesbf")
            nc.vector.memset(ones_bf, 1.0)
            eps_col = consts.tile([P, 1], F32, tag="eps")
            nc.vector.memset(eps_col, EPS)

            wq_sb = consts.tile([P, 2, DK], F32R, tag="wq")
            nc.sync.dma_start(out=wq_sb, in_=wq_ext.rearrange("(h p) d -> p h d", p=P).bitcast(F32R))
            wk_sb = consts.tile([P, 2, DK], F32R, tag="wk")
            nc.sync.dma_start(out=wk_sb, in_=wk_ext.rearrange("(h p) d -> p h d", p=P).bitcast(F32R))
            wtk_sb = consts.tile([P, 2, DK], F32R, tag="wtk")
            nc.sync.dma_start(out=wtk_sb, in_=wtk_ext.rearrange("(h p) d -> p h d", p=P).bitcast(F32R))

            bq_sb = consts.tile([P, 1], F32, tag="bq")
            nc.sync.dma_start(out=bq_sb, in_=bq_ext[:, :])
            bk_sb = consts.tile([P, 1], F32, tag="bk")
            nc.sync.dma_start(out=bk_sb, in_=bk_ext[:, :])
            btk_sb = consts.tile([1, DK], F32R, tag="btk")
            nc.sync.dma_start(out=btk_sb, in_=btk_ext[:, :].bitcast(F32R))
            bmu_sb = consts.tile([1, C], F32R, tag="bmu")
            nc.sync.dma_start(out=bmu_sb, in_=bmu_ext[:, :].bitcast(F32R))
            bsig_sb = consts.tile([1, C], F32R, tag="bsig")
            nc.sync.dma_start(out=bsig_sb, in_=bsig_ext[:, :].bitcast(F32R))
            t_sb = consts.tile([LK, C], F32R, tag="tsb")
            nc.sync.dma_start(out=t_sb, in_=t_ext[:, :].bitcast(F32R))

            # resident block-major x + bf16 exp scores for all pixels
            X = consts.tile([P, P, C], F32R, tag="X")
            E = consts.tile([L33, NPIX], BF16, tag="E")
            # Wmu/Wsig borrow the X tail; consumed in phase 0 before x js 112+
            # arrive (dma_in(7) is issued at i=24)
            wmu_sb = X[:, 112:114, :]
            nc.sync.dma_start(out=wmu_sb, in_=wmu_ext.rearrange("(h p) d -> p h d", p=P).bitcast(F32R))
            wsig_sb = X[:, 114:116, :]
            nc.sync.dma_start(out=wsig_sb, in_=wsig_ext.rearrange("(h p) d -> p h d", p=P).bitcast(F32R))

            tT_sb = consts.tile([P, 2, 32], F32R, tag="tT")
            tkT_sb = consts.tile([P, 32], F32R, tag="tkT")
            mu_t = consts.tile([L1, C], F32, tag="mut")
            sig_t = consts.tile([L1, C], F32, tag="sigt")
            bn_st = consts.tile([P, 2, NT, 6], F32, tag="bnst")
            srs2_sb = consts.tile([L33, C], BF16, tag="srs2")
            m12_sb = consts.tile([L33, C], BF16, tag="m12")
            nc.vector.memset(E, 0.0)
            nc.vector.memset(srs2_sb, 0.0)
            nc.vector.memset(m12_sb, 0.0)

            with (
                tc.tile_pool(name="pstr", bufs=1, space="PSUM") as pstr,
                tc.tile_pool(name="psmm", bufs=2, space="PSUM") as psmm,
                tc.tile_pool(name="psc", bufs=1, space="PSUM") as psc,
            ):
                # -------- input DMA: first two tiles up front, rest rolled --------
                def dma_in(g):
                    nc.sync.dma_start(
                        out=X[:, 16 * g : 16 * g + 16, :],
                        in_=x_blk[:, 16 * g : 16 * g + 16, :].bitcast(F32R),
                    )

                dma_in(0)

                # ---------------- phase 0: text projections ----------------
                trT = pstr.tile([P, 2, TW], F32, tag="tr")
                for h in range(2):
                    nc.tensor.transpose(
                        rr(trT[:, 0, h * 32 : h * 32 + LK]),
                        t_sb[:, h * P : (h + 1) * P],
                        ident[:LK, :LK],
                    )
                nc.scalar.activation(tT_sb[:, 0, :LK], trT[:, 0, 0:LK], AF.Copy)
                nc.scalar.activation(tT_sb[:, 1, :LK], trT[:, 0, 32 : 32 + LK], AF.Copy)

                # t_kT [dk, 18] = Wtk.T @ tT + btk
                tkp = psmm.tile([P, 2 * TW], F32, tag="qk")
                for h in range(2):
                    nc.tensor.matmul(
                        tkp[:, :LK],
                        wtk_sb[:, h, :],
                        tT_sb[:, h, :LK],
                        start=(h == 0),
                        stop=False,
                    )
                nc.tensor.matmul(
                    tkp[:, :LK], btk_sb, ones[:1, :LK], start=False, stop=True
                )
                nc.scalar.activation(tkT_sb[:, :LK], tkp[:, :LK], AF.Copy)

                # mu_t / sigma_t [18, C]
                for dst, wsb, brow in ((mu_t, wmu_sb, bmu_sb), (sig_t, wsig_sb, bsig_sb)):
                    pp = psc.tile([P, TW], F32, tag="sc")
                    for h in range(2):
                        nc.tensor.matmul(
                            pp[:LK, :C],
                            tT_sb[:, h, :LK],
                            wsb[:, h, :],
                            start=(h == 0),
                            stop=False,
                        )
                    nc.tensor.matmul(
                        pp[:LK, :C], ones[:1, :LK], brow, start=False, stop=True
                    )
                    nc.scalar.activation(dst[:LK, :], pp[:LK, :C], AF.Copy)

                # ---------------- pass 1: stream tiles ----------------
                for i in range(NT):
                    if i % 4 == 0 and i // 4 + 1 < 8:
                        dma_in(i // 4 + 1)

                    # channel-major xT via PE transpose (2 c-halves)
                    trx = pstr.tile([P, 2, TW], F32, tag="tr")
                    for m in range(4):
                        nc.tensor.transpose(
                            rr(trx[:, 0, m * P : (m + 1) * P]),
                            X[:, 4 * i + m, 0:P],
                            ident,
                        )
                        nc.tensor.transpose(
                            rr(trx[:, 1, m * P : (m + 1) * P]),
                            X[:, 4 * i + m, P:C],
                            ident,
                        )
                    xT = work.tile([P, 2, TW], F32R, tag="xT")
                    nc.scalar.activation(xT, trx, AF.Copy)
                    xT0 = xT[:, 0, :]
                    xT1 = xT[:, 1, :]

                    # instance-norm stats: fused mean/var per half on DVE
                    nc.vector.bn_stats(bn_st[:, 0, i, :], xT0)
                    nc.vector.bn_stats(bn_st[:, 1, i, :], xT1)

                    # projections x_qT / x_kT [dk, 512] (one psum tile)
                    pqk = psmm.tile([P, 2 * TW], F32, tag="qk")
                    for h in range(2):
                        nc.tensor.matmul(
                            pqk[:, :TW], wq_sb[:, h, :], xT0 if h == 0 else xT1,
                            start=(h == 0), stop=(h == 1),
                        )
                    for h in range(2):
                        nc.tensor.matmul(
                            pqk[:, TW:], wk_sb[:, h, :], xT0 if h == 0 else xT1,
                            start=(h == 0), stop=(h == 1),
                        )
                    xq_sb = work.tile([P, TW], F32R, tag="xq")
                    nc.vector.tensor_scalar_add(xq_sb, pqk[:, :TW], bq_sb[:, :])
                    prod = work.tile([P, TW], F32R, tag="prod")
                    nc.vector.scalar_tensor_tensor(
                        prod, pqk[:, TW:], bk_sb[:, :], xq_sb, op0=OP.add, op1=OP.mult
                    )

                    # token-major scores: cross at rows 0:18, self at row 32
                    # (matmul output base partition must be 0/32/64)
                    sc19 = psc.tile([P, TW], F32, tag="sc")
                    nc.tensor.matmul(
                        sc19[:LK, :], tkT_sb[:, :LK], xq_sb, start=True, stop=True
                    )
                    sc_self = psc.tile([1, TW], F32, tag="self")
                    nc.tensor.matmul(
                        sc_self, ones[:, 0:1], prod, start=True, stop=True
                    )
                    nc.scalar.activation(
                        E[:LK, i * TW : (i + 1) * TW], sc19[:LK, :], AF.Exp
                    )
                    nc.scalar.activation(
                        E[32:33, i * TW : (i + 1) * TW], sc_self, AF.Exp
                    )

            # ---------------- stats finalize ----------------
            with tc.tile_pool(name="psfin", bufs=1, space="PSUM") as psfin:
                agg = scr.tile([P, 2, 2], F32, tag="agg")
                nc.vector.bn_aggr(agg[:, 0, :], bn_st[:, 0, :, :])
                nc.vector.bn_aggr(agg[:, 1, :], bn_st[:, 1, :, :])
                mean2 = agg[:, :, 0]
                var2 = agg[:, :, 1]
                # quad cols: rs_h0, rs_h1, murs_h0, murs_h1
                quad = scr.tile([P, 4], F32R, tag="quad")
                nc.scalar.activation(quad[:, 0:2], var2, AF.Ln, bias=eps_col)
                nc.scalar.activation(quad[:, 0:2], quad[:, 0:2], AF.Exp, scale=-0.5)
                nc.vector.tensor_mul(quad[:, 2:4], mean2, quad[:, 0:2])

                pq4 = psfin.tile([1, 4, P], F32, tag="q4")
                for m in range(4):
                    nc.tensor.transpose(rr(pq4[:, m, :]), quad[:, m : m + 1], ident)
                rows4 = scr.tile([1, 2, C], F32R, tag="rows4")
                nc.scalar.activation(rows4[:, 0, 0:P], pq4[:, 0, :], AF.Copy)
                nc.scalar.activation(rows4[:, 0, P:C], pq4[:, 1, :], AF.Copy)
                nc.scalar.activation(rows4[:, 1, 0:P], pq4[:, 2, :], AF.Copy)
                nc.scalar.activation(rows4[:, 1, P:C], pq4[:, 3, :], AF.Copy)

                # srs2 = [(sig_t+1).*rs ; 1+rs],  m12 = [mu_t-(sig_t+1).*murs ; -murs]
                p_rs = psfin.tile([L33, C], F32, tag="rsrep")
                nc.tensor.matmul(p_rs, ones[:1, :L33], rows4[:, 0, :], start=True, stop=True)
                p_mu = psfin.tile([L33, C], F32, tag="mursrep")
                nc.tensor.matmul(p_mu, ones[:1, :L33], rows4[:, 1, :], start=True, stop=True)

                nc.vector.scalar_tensor_tensor(
                    srs2_sb[:LK, :], sig_t[:LK, :], 1.0, p_rs[:LK, :],
                    op0=OP.add, op1=OP.mult,
                )
                nc.vector.tensor_scalar_add(srs2_sb[32:33, :], p_rs[32:33, :], 1.0)
                tmp18 = scr.tile([L1, C], F32, tag="tmp18")
                nc.vector.scalar_tensor_tensor(
                    tmp18[:LK, :], sig_t[:LK, :], 1.0, p_mu[:LK, :],
                    op0=OP.add, op1=OP.mult,
                )
                nc.vector.tensor_sub(m12_sb[:LK, :], mu_t[:LK, :], tmp18[:LK, :])
                nc.vector.tensor_scalar_mul(m12_sb[32:33, :], p_mu[32:33, :], -1.0)

            # ---------------- pass 2: P/Q matmuls + epilogue ----------------
            with (
                tc.tile_pool(name="psq", bufs=4, space="PSUM") as psq,
                tc.tile_pool(name="psz", bufs=1, space="PSUM") as psz,
                tc.tile_pool(name="ep", bufs=4) as ep,
            ):
                for i in range(NT):
                    pz = psz.tile([P, 4], F32, tag="z")
                    rz = ep.tile([P, 4], F32, tag="rz")
                    pqps = []
                    for m in range(4):
                        ec = E[:, (4 * i + m) * P : (4 * i + m + 1) * P]
                        pqp = psq.tile([P, 2, C], F32, tag="qp")
                        nc.tensor.matmul(pqp[:, 0, :], ec, srs2_sb, start=True, stop=True)
                        nc.tensor.matmul(pqp[:, 1, :], ec, m12_sb, start=True, stop=True)
                        nc.tensor.matmul(
                            pz[:, m : m + 1], ec, ones_bf,
                            start=True, stop=True,
                        )
                        nc.vector.reciprocal(rz[:, m : m + 1], pz[:, m : m + 1])
                        pqps.append(pqp)
                    for m in range(4):
                        if m % 2 == 0:
                            out_sb = outp.tile([P, 2, C], F32, tag="out")
                        v1 = ep.tile([P, C], F32, tag="v1")
                        nc.vector.scalar_tensor_tensor(
                            v1, pqps[m][:, 0, :], rz[:, m : m + 1],
                            X[:, 4 * i + m, :].bitcast(F32),
                            op0=OP.mult, op1=OP.mult,
                        )
                        t2 = ep.tile([P, C], F32, tag="t2")
                        nc.scalar.activation(
                            t2, pqps[m][:, 1, :], AF.Copy, scale=rz[:, m : m + 1]
                        )
                        if m == 3:
                            nc.vector.tensor_add(out_sb[:, m % 2, :], v1, t2)
                        else:
                            nc.gpsimd.tensor_add(out_sb[:, m % 2, :], v1, t2)
                        if m % 2 == 1:
                            nc.sync.dma_start(
                                out=out_blk[:, 4 * i + m - 1 : 4 * i + m + 1, :],
                                in_=out_sb,
                            )

    return nc


_NC_CACHE = None


def _get_nc():
    global _NC_CACHE
    if _NC_CACHE is None:
        _NC_CACHE = _build(bacc.Bacc())
        _NC_CACHE.compile()
    return _NC_CACHE


def kernel(**inputs):
    global LAST_RESULT
    n = inputs["x"].shape[0]
    f = np.ascontiguousarray
    in_maps = []
    for b in range(n):
        in_maps.append(
            {
                "x": f(inputs["x"][b].reshape(NPIX, C).astype(np.float32)),
                "t": f(inputs["t"][b].astype(np.float32)),
                "Wq": f(inputs["Wq"].astype(np.float32)),
                "bq": f(inputs["bq"].astype(np.float32).reshape(DK, 1)),
                "Wk": f(inputs["Wk"].astype(np.float32)),
                "bk": f(inputs["bk"].astype(np.float32).reshape(DK, 1)),
                "Wtk": f(inputs["Wtk"].astype(np.float32)),
                "btk": f(inputs["btk"].astype(np.float32).reshape(1, DK)),
                "Wmu": f(inputs["Wmu"].astype(np.float32)),
                "bmu": f(inputs["bmu"].astype(np.float32).reshape(1, C)),
                "Wsig": f(inputs["Wsig"].astype(np.float32)),
                "bsig": f(inputs["bsig"].astype(np.float32).reshape(1, C)),
            }
        )
    nc = _get_nc()
    res = run_bass_kernel_spmd(
        nc, in_maps, core_ids=list(range(n)), trace=bool(os.environ.get("BASS_TRACE"))
    )
    LAST_RESULT = res
    h = w = 128
    out = np.stack([res.results[b]["out"].reshape(h, w, C) for b in range(n)])
    return out.astype(np.float32)


# revision 23
# speedup vs baseline: 1.9946x; 1.2451x over previous
"""ACAN sparse-attention kernel for 8x TRN2 NeuronCores.

Data-parallel over batch n=8 (one batch image per core, weights replicated).
Per core: x [16384, 256] instance-norm + QK self/cross scores vs 18 text
tokens + per-pixel softmax(19) + weighted mu/sigma modulation.

Math (per pixel p, channel c), with E = exp([x_q@t_k.T | x_q.x_k]) and
Z = sum_k E[k,p], rz = 1/Z:
  out = (1 + ws)*xn + wm + sx*x
      = x .* (rz * (E19.T @ srs2)) + rz * (E19.T @ m12)
  where srs2[k<18] = (sig_t+1).*rs,  srs2[18] = 1 + rs
        m12 [k<18] = mu_t - (sig_t+1).*murs,  m12[18] = -murs
        rs = 1/sqrt(var+eps), murs = mean*rs   (instance-norm stats)
Two-pass streaming: pass 1 loads x, computes scores token-major + exp once
(stored bf16) + stats partials; pass 2 re-expands E via two small PE
matmuls per 128-px chunk and a 2-op elementwise epilogue split across
DVE / Scalar / GpSimd.
"""

import os
import sys

import numpy as np

for _p in ("/opt/trn_rl_repo", "/root/.axon_site/_ro/trn_rl_repo"):
    if os.path.isdir(_p) and _p not in sys.path:
        sys.path.append(_p)

import concourse.bass as bass
import concourse.bacc as bacc
import concourse.tile as tile
from concourse import mybir
from concourse.bass_utils import run_bass_kernel_spmd

F32 = mybir.dt.float32
F32R = mybir.dt.float32r
BF16 = mybir.dt.bfloat16
AF = mybir.ActivationFunctionType
OP = mybir.AluOpType

P = 128          # partitions
C = 256          # channels
DK = 128         # query/key dim
LK = 18          # text tokens
L1 = LK + 1      # tokens + self
L33 = 33         # E rows: 0-17 cross, 18-31 zero, 32 self (ACT partition-base rule)
NPIX = 128 * 128
TW = 512         # pixels per tile
NT = NPIX // TW  # 32 tiles
EPS = 1e-5

LAST_RESULT = None


def rr(ap):
    return ap.bitcast(F32R)


def _build(nc: bass.Bass):
    x_ext = nc.declare_dram_parameter("x", [NPIX, C], F32, isOutput=False)
    t_ext = nc.declare_dram_parameter("t", [LK, C], F32, isOutput=False)
    wq_ext = nc.declare_dram_parameter("Wq", [C, DK], F32, isOutput=False)
    bq_ext = nc.declare_dram_parameter("bq", [DK, 1], F32, isOutput=False)
    wk_ext = nc.declare_dram_parameter("Wk", [C, DK], F32, isOutput=False)
    bk_ext = nc.declare_dram_parameter("bk", [DK, 1], F32, isOutput=False)
    wtk_ext = nc.declare_dram_parameter("Wtk", [C, DK], F32, isOutput=False)
    btk_ext = nc.declare_dram_parameter("btk", [1, DK], F32, isOutput=False)
    wmu_ext = nc.declare_dram_parameter("Wmu", [C, C], F32, isOutput=False)
    bmu_ext = nc.declare_dram_parameter("bmu", [1, C], F32, isOutput=False)
    wsig_ext = nc.declare_dram_parameter("Wsig", [C, C], F32, isOutput=False)
    bsig_ext = nc.declare_dram_parameter("bsig", [1, C], F32, isOutput=False)
    out_ext = nc.declare_dram_parameter("out", [NPIX, C], F32, isOutput=True)

    # block-major pixel layout: partition = pix//128, free t = pix%128
    x_blk = x_ext.rearrange("(p t) c -> p t c", p=P)
    out_blk = out_ext.rearrange("(p t) c -> p t c", p=P)

    with tile.TileContext(nc) as tc:
        with (
            tc.tile_pool(name="consts", bufs=1) as consts,
            tc.tile_pool(name="work", bufs=2) as work,
            tc.tile_pool(name="scr", bufs=1) as scr,
            tc.tile_pool(name="outp", bufs=3) as outp,
        ):
            # ---------------- constants / small inputs ----------------
            from concourse.masks import make_identity
            ident_f = consts.tile([P, P], F32, tag="identf")
            make_identity(nc, ident_f)
            ident = consts.tile([P, P], F32R, tag="ident")
            nc.scalar.activation(ident, ident_f, AF.Copy)
            ones_f = consts.tile([P, 64], F32, tag="onesf")
            nc.vector.memset(ones_f, 1.0)
            ones = consts.tile([P, 64], F32R, tag="ones")
            nc.scalar.activation(ones, ones_f, AF.Copy)
            ones_bf = consts.tile([L33, 1], BF16, tag="onesbf")
            nc.vector.memset(ones_bf, 1.0)
            eps_col = consts.tile([P, 1], F32, tag="eps")
            nc.vector.memset(eps_col, EPS)

            wq_sb = consts.tile([P, 2, DK], F32R, tag="wq")
            nc.sync.dma_start(out=wq_sb, in_=wq_ext.rearrange("(h p) d -> p h d", p=P).bitcast(F32R))
            wk_sb = consts.tile([P, 2, DK], F32R, tag="wk")
            nc.sync.dma_start(out=wk_sb, in_=wk_ext.rearrange("(h p) d -> p h d", p=P).bitcast(F32R))
            wtk_sb = consts.tile([P, 2, DK], F32R, tag="wtk")
            nc.sync.dma_start(out=wtk_sb, in_=wtk_ext.rearrange("(h p) d -> p h d", p=P).bitcast(F32R))

            bq_sb = consts.tile([P, 1], F32, tag="bq")
            nc.sync.dma_start(out=bq_sb, in_=bq_ext[:, :])
            bk_sb = consts.tile([P, 1], F32, tag="bk")
            nc.sync.dma_start(out=bk_sb, in_=bk_ext[:, :])
            btk_sb = consts.tile([1, DK], F32R, tag="btk")
            nc.sync.dma_start(out=btk_sb, in_=btk_ext[:, :].bitcast(F32R))
            bmu_sb = consts.tile([1, C], F32R, tag="bmu")
            nc.sync.dma_start(out=bmu_sb, in_=bmu_ext[:, :].bitcast(F32R))
            bsig_sb = consts.tile([1, C], F32R, tag="bsig")
            nc.sync.dma_start(out=bsig_sb, in_=bsig_ext[:, :].bitcast(F32R))
            t_sb = consts.tile([LK, C], F32R, tag="tsb")
            nc.sync.dma_start(out=t_sb, in_=t_ext[:, :].bitcast(F32R))

            # resident block-major x + bf16 exp scores for all pixels
            X = consts.tile([P, P, C], F32R, tag="X")
            E = consts.tile([L33, NPIX], BF16, tag="E")
            # Wmu/Wsig borrow the X tail; consumed in phase 0 before x js 112+
            # arrive (dma_in(7) is issued at i=24)
            wmu_sb = X[:, 112:114, :]
            nc.sync.dma_start(out=wmu_sb, in_=wmu_ext.rearrange("(h p) d -> p h d", p=P).bitcast(F32R))
            wsig_sb = X[:, 114:116, :]
            nc.sync.dma_start(out=wsig_sb, in_=wsig_ext.rearrange("(h p) d -> p h d", p=P).bitcast(F32R))

            tT_sb = consts.tile([P, 2, 32], F32R, tag="tT")
            tkT_sb = consts.tile([P, 32], F32R, tag="tkT")
            mu_t = consts.tile([L1, C], F32, tag="mut")
            sig_t = consts.tile([L1, C], F32, tag="sigt")
            bn_st = consts.tile([P, 2, NT, 6], F32, tag="bnst")
            srs2_sb = consts.tile([L33, C], BF16, tag="srs2")
            m12_sb = consts.tile([L33, C], BF16, tag="m12")
            nc.gpsimd.memset(srs2_sb, 0.0)
            nc.gpsimd.memset(m12_sb, 0.0)

            with (
                tc.tile_pool(name="pstr", bufs=1, space="PSUM") as pstr,
                tc.tile_pool(name="psmm", bufs=2, space="PSUM") as psmm,
                tc.tile_pool(name="psc", bufs=1, space="PSUM") as psc,
            ):
                # -------- input DMA: fine-grained head, then 16-j groups --------
                def dma_in_js(j0, j1):
                    nc.sync.dma_start(
                        out=X[:, j0:j1, :],
                        in_=x_blk[:, j0:j1, :].bitcast(F32R),
                    )

                for t in range(4):
                    dma_in_js(4 * t, 4 * t + 4)

                # ---------------- phase 0: text projections ----------------
                trT = pstr.tile([P, 2, TW], F32, tag="tr")
                for h in range(2):
                    nc.tensor.transpose(
                        rr(trT[:, 0, h * 32 : h * 32 + LK]),
                        t_sb[:, h * P : (h + 1) * P],
                        ident[:LK, :LK],
                    )
                nc.scalar.activation(tT_sb[:, 0, :LK], trT[:, 0, 0:LK], AF.Copy)
                nc.scalar.activation(tT_sb[:, 1, :LK], trT[:, 0, 32 : 32 + LK], AF.Copy)

                # t_kT [dk, 18] = Wtk.T @ tT + btk
                tkp = psmm.tile([P, 2 * TW], F32, tag="qk")
                for h in range(2):
                    nc.tensor.matmul(
                        tkp[:, :LK],
                        wtk_sb[:, h, :],
                        tT_sb[:, h, :LK],
                        start=(h == 0),
                        stop=False,
                    )
                nc.tensor.matmul(
                    tkp[:, :LK], btk_sb, ones[:1, :LK], start=False, stop=True
                )
                nc.scalar.activation(tkT_sb[:, :LK], tkp[:, :LK], AF.Copy)

                # mu_t / sigma_t [18, C]
                for dst, wsb, brow in ((mu_t, wmu_sb, bmu_sb), (sig_t, wsig_sb, bsig_sb)):
                    pp = psc.tile([P, TW], F32, tag="sc")
                    for h in range(2):
                        nc.tensor.matmul(
                            pp[:LK, :C],
                            tT_sb[:, h, :LK],
                            wsb[:, h, :],
                            start=(h == 0),
                            stop=False,
                        )
                    nc.tensor.matmul(
                        pp[:LK, :C], ones[:1, :LK], brow, start=False, stop=True
                    )
                    nc.scalar.activation(dst[:LK, :], pp[:LK, :C], AF.Copy)

                # ---------------- pass 1: stream tiles ----------------
                for i in range(NT):
                    if i % 4 == 0 and i // 4 + 1 < 8:
                        g = i // 4 + 1
                        dma_in_js(16 * g, 16 * g + 16)
                    # zero this tile's E column (rows 18-31 stay 0; exps overwrite the rest)
                    nc.gpsimd.memset(E[:, i * TW : (i + 1) * TW], 0.0)

                    # channel-major xT via PE transpose (2 c-halves)
                    trx = pstr.tile([P, 2, TW], F32, tag="tr")
                    for m in range(4):
                        nc.tensor.transpose(
                            rr(trx[:, 0, m * P : (m + 1) * P]),
                            X[:, 4 * i + m, 0:P],
                            ident,
                        )
                        nc.tensor.transpose(
                            rr(trx[:, 1, m * P : (m + 1) * P]),
                            X[:, 4 * i + m, P:C],
                            ident,
                        )
                    xT = work.tile([P, 2, TW], F32R, tag="xT")
                    nc.scalar.activation(xT, trx, AF.Copy)
                    xT0 = xT[:, 0, :]
                    xT1 = xT[:, 1, :]

                    # instance-norm stats: fused mean/var per half on DVE
                    nc.vector.bn_stats(bn_st[:, 0, i, :], xT0)
                    nc.vector.bn_stats(bn_st[:, 1, i, :], xT1)

                    # projections x_qT / x_kT [dk, 512] (one psum tile)
                    pqk = psmm.tile([P, 2 * TW], F32, tag="qk")
                    for h in range(2):
                        nc.tensor.matmul(
                            pqk[:, :TW], wq_sb[:, h, :], xT0 if h == 0 else xT1,
                            start=(h == 0), stop=(h == 1),
                        )
                    for h in range(2):
                        nc.tensor.matmul(
                            pqk[:, TW:], wk_sb[:, h, :], xT0 if h == 0 else xT1,
                            start=(h == 0), stop=(h == 1),
                        )
                    xq_sb = work.tile([P, TW], F32R, tag="xq")
                    nc.vector.tensor_scalar_add(xq_sb, pqk[:, :TW], bq_sb[:, :])
                    prod = work.tile([P, TW], F32R, tag="prod")
                    nc.vector.scalar_tensor_tensor(
                        prod, pqk[:, TW:], bk_sb[:, :], xq_sb, op0=OP.add, op1=OP.mult
                    )

                    # token-major scores: cross at rows 0:18, self at row 32
                    # (matmul output base partition must be 0/32/64)
                    sc19 = psc.tile([P, TW], F32, tag="sc")
                    nc.tensor.matmul(
                        sc19[:LK, :], tkT_sb[:, :LK], xq_sb, start=True, stop=True
                    )
                    sc_self = psc.tile([1, TW], F32, tag="self")
                    nc.tensor.matmul(
                        sc_self, ones[:, 0:1], prod, start=True, stop=True
                    )
                    nc.scalar.activation(
                        E[:LK, i * TW : (i + 1) * TW], sc19[:LK, :], AF.Exp
                    )
                    nc.scalar.activation(
                        E[32:33, i * TW : (i + 1) * TW], sc_self, AF.Exp
                    )

            # ---------------- stats finalize ----------------
            with tc.tile_pool(name="psfin", bufs=1, space="PSUM") as psfin:
                agg = scr.tile([P, 2, 2], F32, tag="agg")
                nc.vector.bn_aggr(agg[:, 0, :], bn_st[:, 0, :, :])
                nc.vector.bn_aggr(agg[:, 1, :], bn_st[:, 1, :, :])
                mean2 = agg[:, :, 0]
                var2 = agg[:, :, 1]
                # quad cols: rs_h0, rs_h1, murs_h0, murs_h1
                quad = scr.tile([P, 4], F32R, tag="quad")
                nc.scalar.activation(quad[:, 0:2], var2, AF.Ln, bias=eps_col)
                nc.scalar.activation(quad[:, 0:2], quad[:, 0:2], AF.Exp, scale=-0.5)
                nc.vector.tensor_mul(quad[:, 2:4], mean2, quad[:, 0:2])

                pq4 = psfin.tile([1, 4, P], F32, tag="q4")
                for m in range(4):
                    nc.tensor.transpose(rr(pq4[:, m, :]), quad[:, m : m + 1], ident)
                rows4 = scr.tile([1, 2, C], F32R, tag="rows4")
                nc.scalar.activation(rows4[:, 0, 0:P], pq4[:, 0, :], AF.Copy)
                nc.scalar.activation(rows4[:, 0, P:C], pq4[:, 1, :], AF.Copy)
                nc.scalar.activation(rows4[:, 1, 0:P], pq4[:, 2, :], AF.Copy)
                nc.scalar.activation(rows4[:, 1, P:C], pq4[:, 3, :], AF.Copy)

                # srs2 = [(sig_t+1).*rs ; 1+rs],  m12 = [mu_t-(sig_t+1).*murs ; -murs]
                p_rs = psfin.tile([L33, C], F32, tag="rsrep")
                nc.tensor.matmul(p_rs, ones[:1, :L33], rows4[:, 0, :], start=True, stop=True)
                p_mu = psfin.tile([L33, C], F32, tag="mursrep")
                nc.tensor.matmul(p_mu, ones[:1, :L33], rows4[:, 1, :], start=True, stop=True)

                nc.vector.scalar_tensor_tensor(
                    srs2_sb[:LK, :], sig_t[:LK, :], 1.0, p_rs[:LK, :],
                    op0=OP.add, op1=OP.mult,
                )
                nc.vector.tensor_scalar_add(srs2_sb[32:33, :], p_rs[32:33, :], 1.0)
                tmp18 = scr.tile([L1, C], F32, tag="tmp18")
                nc.vector.scalar_tensor_tensor(
                    tmp18[:LK, :], sig_t[:LK, :], 1.0, p_mu[:LK, :],
                    op0=OP.add, op1=OP.mult,
                )
                nc.vector.tensor_sub(m12_sb[:LK, :], mu_t[:LK, :], tmp18[:LK, :])
                nc.vector.tensor_scalar_mul(m12_sb[32:33, :], p_mu[32:33, :], -1.0)

            # ---------------- pass 2: P/Q matmuls + epilogue ----------------
            with (
                tc.tile_pool(name="psq", bufs=4, space="PSUM") as psq,
                tc.tile_pool(name="psz", bufs=1, space="PSUM") as psz,
                tc.tile_pool(name="ep", bufs=4) as ep,
            ):
                for i in range(NT):
                    pz = psz.tile([P, 4], F32, tag="z")
                    rz = ep.tile([P, 4], F32, tag="rz")
                    pqps = []
                    for m in range(4):
                        ec = E[:, (4 * i + m) * P : (4 * i + m + 1) * P]
                        pqp = psq.tile([P, 2, C], F32, tag="qp")
                        nc.tensor.matmul(pqp[:, 0, :], ec, srs2_sb, start=True, stop=True)
                        nc.tensor.matmul(pqp[:, 1, :], ec, m12_sb, start=True, stop=True)
                        nc.tensor.matmul(
                            pz[:, m : m + 1], ec, ones_bf,
                            start=True, stop=True,
                        )
                        pqps.append(pqp)
                    nc.vector.reciprocal(rz, pz)
                    for m in range(4):
                        if m % 2 == 0:
                            out_sb = outp.tile([P, 2, C], F32, tag="out")
                        v1 = ep.tile([P, C], F32, tag="v1")
                        nc.vector.scalar_tensor_tensor(
                            v1, pqps[m][:, 0, :], rz[:, m : m + 1],
                            X[:, 4 * i + m, :].bitcast(F32),
                            op0=OP.mult, op1=OP.mult,
                        )
                        t2 = ep.tile([P, C], F32, tag="t2")
                        nc.scalar.activation(
                            t2, pqps[m][:, 1, :], AF.Copy, scale=rz[:, m : m + 1]
                        )
                        if m == 3:
                            nc.vector.tensor_add(out_sb[:, m % 2, :], v1, t2)
                        else:
                            nc.gpsimd.tensor_add(out_sb[:, m % 2, :], v1, t2)
                        if m % 2 == 1:
                            nc.sync.dma_start(
                                out=out_blk[:, 4 * i + m - 1 : 4 * i + m + 1, :],
                                in_=out_sb,
                            )

    return nc


_NC_CACHE = None


def _get_nc():
    global _NC_CACHE
    if _NC_CACHE is None:
        _NC_CACHE = _build(bacc.Bacc())
        _NC_CACHE.compile()
    return _NC_CACHE


def kernel(**inputs):
    global LAST_RESULT
    n = inputs["x"].shape[0]
    f = np.ascontiguousarray
    in_maps = []
    for b in range(n):
        in_maps.append(
            {
                "x": f(inputs["x"][b].reshape(NPIX, C).astype(np.float32)),
                "t": f(inputs["t"][b].astype(np.float32)),
                "Wq": f(inputs["Wq"].astype(np.float32)),
                "bq": f(inputs["bq"].astype(np.float32).reshape(DK, 1)),
                "Wk": f(inputs["Wk"].astype(np.float32)),
                "bk": f(inputs["bk"].astype(np.float32).reshape(DK, 1)),
                "Wtk": f(inputs["Wtk"].astype(np.float32)),
                "btk": f(inputs["btk"].astype(np.float32).reshape(1, DK)),
                "Wmu": f(inputs["Wmu"].astype(np.float32)),
                "bmu": f(inputs["bmu"].astype(np.float32).reshape(1, C)),
                "Wsig": f(inputs["Wsig"].astype(np.float32)),
                "bsig": f(inputs["bsig"].astype(np.float32).reshape(1, C)),
            }
        )
    nc = _get_nc()
    res = run_bass_kernel_spmd(
        nc, in_maps, core_ids=list(range(n)), trace=bool(os.environ.get("BASS_TRACE"))
    )
    LAST_RESULT = res
    h = w = 128
    out = np.stack([res.results[b]["out"].reshape(h, w, C) for b in range(n)])
    return out.astype(np.float32)
